# revision 33
# baseline (speedup 1.0000x reference)
# Trainium2 Bass kernel for nn_MultiHeadTransformer (B=2, S=2048, E=1024, H=16, FF=4096).
#
# Sharding: 8-way head/row parallel with zero collectives (the reference's
# "faithful raw view" makes qkv/attention/proj/LN/FFN row-local per core: core c
# computes the qkv rows covering the six flat (type, head) blocks of its two
# heads; the inverse view maps head outputs back to its own 256 token rows).
#
# v5: fp8-e4m3 + DoubleRow on QKV / AV / proj (validated end-to-end rel err
# ~4e-3 vs the 2e-2 gate in a numpy quantization sim; FFN stays fp16 because
# fp8 there alone costs ~4e-2):
#  - QKV: x^T and w_qkv host-packed as [P, 4, 2, *] fp8 kc-pairs; per-mt waves
#    of 6 PSUM accs, 4 DoubleRow passes each; per-g contiguous DRAM slabs so
#    the first matmul starts as soon as slab g0 lands.
#  - scratch (qkv scatter through DRAM) in fp8: half the roundtrip bytes.
#  - q/k staged fp8, PE-transposed fp8, converted to fp8 qT/kT via the
#    existing DVE copy-out; scores run fp8 (same PE rate as bf16) with the
#    two heads' score matmuls emitted interleaved on disjoint PE row halves
#    (rows 0-63 / 64-127) so they execute concurrently.
#  - AV: v staged as [P, 8, 2, 160] fp8 j-pairs (ones row at +64 for the
#    softmax denominator); exp writes fp8 into paired a2 tiles; AV runs one
#    DoubleRow matmul per j-pair, plus a single-j matmul for the causal
#    straddle chunk [s_even, s_odd).
#  - proj: hT scatter written fp8 in kc-pairs, w_proj fp8 [P, 4, 2, E],
#    4 DoubleRow passes per 512-col chunk.
#  - causal masks multiplied on DVE (fp8); gpsimd keeps only scratch writes,
#    xr loads and output writes; output written per 512-col half.
import numpy as np

B, S, E, H, DH, FF = 2, 2048, 1024, 16, 64, 4096
ROW = 3 * E            # 3072 qkv columns
BLK = S * DH           # 131072 elements per (type, head) block
NCORES = 8
P = 128
INV_SCALE = 1.0 / float(np.sqrt(E))

_cached = {}


def _build():
    import concourse.bacc as bacc
    import concourse.bass as bass
    import concourse.mybir as mybir
    import concourse.tile as tile
    from concourse.masks import make_identity

    f32 = mybir.dt.float32
    bf16 = mybir.dt.bfloat16
    fp16 = mybir.dt.float16
    f8 = mybir.dt.float8e4
    u32 = mybir.dt.uint32
    AF = mybir.ActivationFunctionType
    ALU = mybir.AluOpType
    DR = mybir.MatmulPerfMode.DoubleRow

    nc = bacc.Bacc(trn_type="TRN2", target_bir_lowering=False, debug=False,
                   num_devices=NCORES)

    xqT_d = nc.dram_tensor("xqT", [P, 4, 2, 544], f8,
                           kind="ExternalInput").ap()
    wq_d = nc.dram_tensor("wq", [P, 4, 2, ROW], f8, kind="ExternalInput").ap()
    bq_d = nc.dram_tensor("bq", [1, ROW], bf16, kind="ExternalInput").ap()
    wp_d = nc.dram_tensor("wp", [P, 4, 2, E], f8, kind="ExternalInput").ap()
    w1_d = nc.dram_tensor("w1", [P, 8, FF], fp16, kind="ExternalInput").ap()
    b1T_d = nc.dram_tensor("b1T", [P, 32], f32, kind="ExternalInput").ap()
    w2_d = nc.dram_tensor("w2", [P, 32, E], fp16, kind="ExternalInput").ap()
    b2_d = nc.dram_tensor("b2", [1, E], fp16, kind="ExternalInput").ap()
    xr_d = nc.dram_tensor("xr", [B, 2, P, E], f32, kind="ExternalInput").ap()
    offs_d = nc.dram_tensor("offs", [1, 4], u32, kind="ExternalInput").ap()
    triu_d = nc.dram_tensor("triu", [P, P], f8, kind="ExternalInput").ap()
    ones_d = nc.dram_tensor("ones", [1, P], f32, kind="ExternalInput").ap()
    out_d = nc.dram_tensor("out", [B, 2, P, E], f32, kind="ExternalOutput").ap()

    slots = [(b, t) for b in range(B) for t in range(3)]

    with tile.TileContext(nc) as tc:
        with tc.tile_pool(name="singles", bufs=1) as singles, \
             tc.tile_pool(name="dram", bufs=1, space="DRAM") as dram:
            triu = singles.tile([P, P], f8)
            b2_row = singles.tile([1, E], fp16)
            b1T_sb = singles.tile([P, 32], f32)
            ones_f = singles.tile([1, P], f32)
            nc.sync.dma_start(ones_f, ones_d)
            ones_h = singles.tile([1, P], fp16)
            nc.vector.tensor_copy(ones_h, ones_f)
            ident_b = singles.tile([P, P], bf16)
            make_identity(nc, ident_b)
            ident_8 = singles.tile([P, P], f8)
            nc.vector.tensor_copy(ident_8, ident_b)
            ident_h = singles.tile([P, P], fp16)
            make_identity(nc, ident_h)
            eps_t = singles.tile([P, 1], f32)
            nc.vector.memset(eps_t, 1e-5)
            offs_sb = singles.tile([1, 4], u32)
            nc.sync.dma_start(offs_sb, offs_d)
            # offsets are only used by gpsimd (Pool) scratch-write DMAs;
            # loading on all engines costs a ~5us all-engine barrier preamble.
            off_v = [nc.values_load(offs_sb[:, t:t + 1],
                                    engines=[mybir.EngineType.Pool],
                                    min_val=0, max_val=ROW,
                                    skip_runtime_bounds_check=True)
                     for t in range(3)]

            SCR88 = ROW + 88 * ROW
            scr = [[dram.tile([SCR88], f8, tag=f"scr{b}{t}",
                              name=f"scr{b}_{t}") for t in range(3)]
                   for b in range(B)]

            # PSUM pools are phase-scoped: psA (QKV accs + q/k transposes)
            # -> psB (attention sc+oT, 8 banks) -> psC (proj accs).
            psA_cm = tc.tile_pool(name="psA", bufs=1, space="PSUM")
            psA = psA_cm.__enter__()
            midpool_cm = tc.tile_pool(name="midpool", bufs=1, side="right")
            midpool = midpool_cm.__enter__()
            hT = midpool.tile([P, 4, 4, 2, P], f8)
            lnT = midpool.tile([P, 8, 4, P], fp16)
            h1T = midpool.tile([P, 32, 4, P], fp16)
            atn_cm = tc.tile_pool(name="atn", bufs=1, side="right")
            atn = atn_cm.__enter__()

            qs8, ks8, vv, qT, kT = {}, {}, {}, {}, {}

            def emit_read(b):
                # merged scratch reads: one DMA per tensor (q/k/v), both
                # heads, all 16 token blocks.  Issued on sync.
                qs8[b] = atn.tile([P, 16, P], f8, tag="qstage", bufs=2,
                                  name=f"qs{b}")
                ks8[b] = atn.tile([P, 16, P], f8, tag="kstage", bufs=2,
                                  name=f"ks{b}")
                vv[b] = atn.tile([P, 8, 2, 160], f8, tag="v", bufs=2,
                                 name=f"v{b}")
                nc.vector.memset(vv[b][:, :, :, 64:65], 1.0)
                nc.vector.memset(vv[b][:, :, :, 144:145], 1.0)
                def src_of(t, hh):
                    return (scr[b][t][ROW + hh * BLK:ROW + (hh + 1) * BLK]
                            .rearrange("(i p d) -> p i d", p=P, d=DH))
                # q first (its transposes gate the first scores), then k, v
                for hh in range(2):
                    nc.sync.dma_start(
                        qs8[b][:, :, 64 * hh:64 * hh + 64], src_of(0, hh))
                for hh in range(2):
                    nc.sync.dma_start(
                        ks8[b][:, :, 64 * hh:64 * hh + 64], src_of(1, hh))
                for hh in range(2):
                    vdst = vv[b][:, :, :, 80 * hh:80 * hh + 64]
                    nc.sync.dma_start(
                        vdst.rearrange("p g j d -> p (g j) d"), src_of(2, hh))
                qT[b] = atn.tile([P, S], f8, tag="qT", bufs=2,
                                 name=f"qT{b}")
                kT[b] = atn.tile([P, S], f8, tag="kT", bufs=2,
                                 name=f"kT{b}")

            def emit_tp(b):
                # all q transposes first: the first scores only need qT+kT[0]
                for st, dst in ((qs8[b], qT[b]), (ks8[b], kT[b])):
                    for i in range(16):
                        # fp8 PE transpose writes 2-byte slots: use a
                        # stride-2 output AP (low byte of each pair).
                        t_ps = psA.tile([P, 1024, 2], f8, tag="tp", bufs=2,
                                        name=f"tp{b}_{i}")
                        nc.tensor.transpose(t_ps[:, 0:P, 0], st[:, i, :],
                                            ident_8)
                        nc.vector.tensor_copy(dst[:, P * i:P * (i + 1)],
                                              t_ps[:, 0:P, 0])

            # ---------------- Phase A: QKV fp8 DoubleRow ------------------
            poolA_cm = tc.tile_pool(name="poolA", bufs=1)
            poolA = poolA_cm.__enter__()
            wq_sb = poolA.tile([P, 4, 2, ROW], f8)
            xqT_sb = poolA.tile([P, 4, 2, 544], f8)
            bias_bc = poolA.tile([P, ROW], bf16)
            # per-g slabs, g0 first so matmuls start ~3us in; wq slab split
            # in halves so wave g can begin on its first columns.
            for g in range(4):
                nc.sync.dma_start(xqT_sb[:, g], xqT_d[:, g])
                nc.sync.dma_start(wq_sb[:, g, :, 0:1536],
                                  wq_d[:, g, :, 0:1536])
                nc.sync.dma_start(wq_sb[:, g, :, 1536:ROW],
                                  wq_d[:, g, :, 1536:ROW])
            nc.sync.dma_start(bias_bc, bq_d.to_broadcast([P, ROW]))
            nc.sync.dma_start(triu, triu_d)
            nc.sync.dma_start(b2_row, b2_d)
            nc.sync.dma_start(b1T_sb, b1T_d)

            # 528 packed rows -> 5 M-tiles of <=128; slot m=(b,t) owns
            # global rows [88m, 88m+88).
            yts = {}
            for mt in range(5):
                g0, g1 = 128 * mt, min(128 * mt + 128, 528)
                M = g1 - g0
                y = poolA.tile([P, ROW], f8, tag="y", bufs=2,
                               name=f"y{mt}")
                yts[mt] = y
                accs = [psA.tile([P, 512], f32, tag="acc", bufs=6,
                                 name=f"qa{mt}_{n6}") for n6 in range(6)]
                for g in range(4):
                    for n6 in range(6):
                        ns = slice(n6 * 512, (n6 + 1) * 512)
                        nc.tensor.matmul(accs[n6][:M, :],
                                         lhsT=xqT_sb[:, g, :, g0:g1],
                                         rhs=wq_sb[:, g, :, ns],
                                         start=(g == 0), stop=(g == 3),
                                         perf_mode=DR)
                for n6 in range(6):
                    ns = slice(n6 * 512, (n6 + 1) * 512)
                    nc.vector.tensor_add(y[:M, ns], accs[n6][:M, :],
                                         bias_bc[:M, ns])
                for m, (b, t) in enumerate(slots):
                    s0, s1 = 88 * m, 88 * m + 88
                    if mt != (s1 - 1) // 128:
                        continue
                    # rows of this slot, split by containing tile
                    spans = []
                    for tt in (mt - 1, mt):
                        if tt < 0:
                            continue
                        lo = max(s0, 128 * tt)
                        hi = min(s1, 128 * tt + 128)
                        if lo < hi:
                            spans.append((tt, lo, hi))
                    dst = scr[b][t][bass.ds(off_v[t], 88 * ROW)]
                    dst_r = dst.rearrange("(r c) -> r c", c=ROW)
                    for tt, lo, hi in spans:
                        nc.gpsimd.dma_start(
                            dst_r[lo - s0:hi - s0, :],
                            yts[tt][lo - 128 * tt:hi - 128 * tt, :])
                    if m == 2:
                        emit_read(0)
            poolA_cm.__exit__(None, None, None)

            w1pool_cm = tc.tile_pool(name="w1pool", bufs=1)
            w1pool = w1pool_cm.__enter__()
            w1_sb = w1pool.tile([P, 8, FF], fp16)
            wppool_cm = tc.tile_pool(name="wppool", bufs=1)
            wppool = wppool_cm.__enter__()
            wp_sb = wppool.tile([P, 4, 2, E], f8)
            emit_read(1)
            emit_tp(0)
            emit_tp(1)
            nc.sync.dma_start(wp_sb, wp_d)
            nc.sync.dma_start(w1_sb[:, 0:2, :], w1_d[:, 0:2, :])
            # residual inputs preloaded on the gpsimd queue (idle here) so
            # they are never stuck behind the w2 slabs later.
            xr_sbs = {}
            for mi in range(4):
                xr_sbs[mi] = atn.tile([P, E], f32, tag="xr", bufs=4,
                                      name=f"xr{mi}")
                nc.gpsimd.dma_start(xr_sbs[mi], xr_d[mi // 2, mi % 2])
            psA_cm.__exit__(None, None, None)
            psB_cm = tc.tile_pool(name="psB", bufs=1, space="PSUM")
            psB = psB_cm.__enter__()

            # ------------- Phase B+C: attention / proj / LN ---------------
            # Query-quarter blocks (512 queries): both heads' scores share
            # one 2-bank PSUM tile, so exp runs ONCE per j for both heads
            # (halves ACT instruction overhead); sc bufs=3 gives a 2-j
            # lookahead so ACT never starves; AV per j-pair via DoubleRow.
            oT_sbs, dnrows, wraps = {}, {}, {}

            def emit_pair(b):
                for hh in range(2):
                    mi = 2 * b + hh
                    oT_sbs[mi] = atn.tile([64, S], bf16, tag="oTsb",
                                          bufs=2, name=f"oTsb{mi}")
                    dnrows[mi] = atn.tile([1, S], bf16, tag="dnrow",
                                          bufs=2, name=f"dn{mi}")
                    wraps[mi] = atn.tile([P, 16], bf16, tag="wrap",
                                         bufs=2, name=f"wrap{mi}")

                for Qq in range(4):
                    Q0 = 512 * Qq
                    jmax = 4 * (Qq + 1)
                    oT = {hh: psB.tile([65, 512], f32, tag="oT", bufs=2,
                                       name=f"oT{b}{hh}_{Qq}")
                          for hh in range(2)}

                    def emit_scores(j):
                        s = max(128 * j - Q0, 0)
                        t_sc = psB.tile([P, 2, 512], f32, tag="sc",
                                        bufs=3, name=f"sc{b}_{Qq}_{j}")
                        for hh in range(2):
                            hp = slice(64 * hh, 64 * hh + 64)
                            nc.tensor.matmul(
                                t_sc[:, hh, s:512],
                                lhsT=kT[b][hp, P * j:P * (j + 1)],
                                rhs=qT[b][hp, Q0 + s:Q0 + 512],
                                start=True, stop=True)
                        return t_sc

                    sc = emit_scores(0)
                    a2 = None
                    for j in range(jmax):
                        gp, jo = j // 2, j % 2
                        s = max(128 * j - Q0, 0)
                        if jo == 0:
                            a2 = atn.tile([P, 2, 2, 512], f8, tag="a",
                                          bufs=4, name=f"a{b}_{Qq}_{gp}")
                        nc.scalar.activation(a2[:, jo, :, s:512],
                                             sc[:, :, s:512],
                                             AF.Exp,
                                             scale=float(INV_SCALE))
                        if 128 * j >= Q0:
                            for hh in range(2):
                                nc.vector.tensor_mul(
                                    a2[:, jo, hh, s:s + P],
                                    a2[:, jo, hh, s:s + P], triu)
                        if j + 1 < jmax:
                            sc = emit_scores(j + 1)
                        if jo == 1:
                            s0 = max(128 * (j - 1) - Q0, 0)
                            s1 = s
                            for hh in range(2):
                                if s1 > s0:
                                    nc.tensor.matmul(
                                        oT[hh][:, s0:s1],
                                        lhsT=vv[b][:, gp, 0,
                                                   80 * hh:80 * hh + 65],
                                        rhs=a2[:, 0, hh, s0:s1],
                                        start=(gp == 0), stop=False)
                                nc.tensor.matmul(
                                    oT[hh][:, s1:512],
                                    lhsT=vv[b][:, gp, :,
                                               80 * hh:80 * hh + 65],
                                    rhs=a2[:, :, hh, s1:512],
                                    start=(gp == 0),
                                    stop=(gp == 2 * Qq + 1),
                                    perf_mode=DR)
                    for hh in range(2):
                        mi = 2 * b + hh
                        nc.vector.tensor_copy(
                            oT_sbs[mi][:, Q0:Q0 + 512], oT[hh][0:64, :])
                        nc.vector.tensor_copy(
                            dnrows[mi][:, Q0:Q0 + 512], oT[hh][64:65, :])
                        dnd_fq = dram.tile([1, 512], bf16, tag="dndf",
                                           bufs=8, name=f"dndf{mi}_{Qq}")
                        nc.sync.dma_start(dnd_fq,
                                          dnrows[mi][:, Q0:Q0 + 512])
                        nc.sync.dma_start(
                            wraps[mi][32 * Qq:32 * Qq + 32, :],
                            dnd_fq[0, :].rearrange("(p f) -> p f", f=16))

            def emit_norm(mi):
                b, hh = mi // 2, mi % 2
                oT_sb = oT_sbs.pop(mi)
                dnrows.pop(mi)
                wrap = wraps.pop(mi)
                wrap_b = atn.tile([P, 16], bf16, tag="wrapb", bufs=2,
                                  name=f"wrapb{mi}")
                with nc.allow_low_precision(reason="softmax denom recip"):
                    nc.vector.reciprocal(wrap_b, wrap)
                dnd_b = dram.tile([1, S], bf16, tag="dndb", bufs=2,
                                  name=f"dndb{mi}")
                nc.sync.dma_start(
                    dnd_b[0, :].rearrange("(p f) -> p f", f=16), wrap_b)
                rrep = atn.tile([64, S], bf16, tag="rrep", bufs=2,
                                name=f"rrep{mi}")
                nc.sync.dma_start(rrep, dnd_b.to_broadcast([64, S]))
                # normalization folded into the proj-lhsT scatter (fp8 out)
                oT_r = oT_sb.rearrange("d (t a) -> d a t", a=16)
                rr_r = rrep.rearrange("d (t a) -> d a t", a=16)
                # mi0/1: half the muls on gpsimd so the DVE FIFO block in
                # front of b1's masks halves; mi2/3 post-attention stay DVE
                # (keeps the gpsimd queue clear to issue the w2 slabs).
                for kc in range(8):
                    for ah in range(2):
                        eng = nc.gpsimd if mi < 2 else nc.vector
                        eng.tensor_mul(
                            hT[64 * ah:64 * ah + 64, mi, kc // 2, kc % 2, :],
                            oT_r[:, 2 * kc + ah, :],
                            rr_r[:, 2 * kc + ah, :])

            def emit_proj(mi):
                b, hh = mi // 2, mi % 2
                xr_sb = xr_sbs[mi]
                r_sb = atn.tile([P, E], f32, tag="r", bufs=2,
                                name=f"r{mi}")
                for ns_i in range(2):
                    ns = slice(ns_i * 512, (ns_i + 1) * 512)
                    pacc = psC.tile([P, 512], f32, tag="pacc", bufs=2,
                                    name=f"pa{mi}_{ns_i}")
                    for g in range(4):
                        nc.tensor.matmul(pacc, lhsT=hT[:, mi, g, :, :],
                                         rhs=wp_sb[:, g, :, ns],
                                         start=(g == 0),
                                         stop=(g == 3),
                                         perf_mode=DR)
                    nc.vector.tensor_add(r_sb[:, ns], pacc,
                                         xr_sb[:, ns])
                stats = atn.tile([P, 2, 6], f32, tag="stats", bufs=2,
                                 name=f"st{mi}")
                for sg in range(2):
                    nc.vector.bn_stats(stats[:, sg, :],
                                       r_sb[:, sg * 512:(sg + 1) * 512])
                mv = atn.tile([P, 2], f32, tag="mv", bufs=2,
                              name=f"mv{mi}")
                nc.vector.bn_aggr(mv, stats)
                nc.scalar.activation(mv[:, 1:2], mv[:, 1:2], AF.Sqrt,
                                     bias=eps_t, scale=1.0)
                nc.vector.reciprocal(mv[:, 1:2], mv[:, 1:2])
                ln_m = atn.tile([P, E], fp16, tag="ln", bufs=2,
                                name=f"ln{mi}")
                nc.vector.tensor_scalar(ln_m, r_sb, mv[:, 0:1],
                                        mv[:, 1:2], ALU.subtract,
                                        ALU.mult)
                for kc in range(8):
                    t_ps = psC.tile([P, 1024], fp16, tag="tpl", bufs=2,
                                    name=f"tpl{mi}_{kc}")
                    nc.tensor.transpose(t_ps[:, 0:P],
                                        ln_m[:, P * kc:P * (kc + 1)],
                                        ident_h)
                    nc.vector.tensor_copy(lnT[:, kc, mi, :], t_ps[:, 0:P])

            emit_pair(0)
            nc.sync.dma_start(w1_sb[:, 2:5, :], w1_d[:, 2:5, :])
            emit_norm(0)
            emit_norm(1)
            emit_pair(1)
            nc.sync.dma_start(w1_sb[:, 5:8, :], w1_d[:, 5:8, :])
            psB_cm.__exit__(None, None, None)
            # proj (4 banks) + FFN1 accs (2 banks) coexist: FFN1 is split by
            # mi-pair (N=256) and interleaved with the proj emissions so the
            # PE never idles >3.4us here (HAM stays at full clock).
            psC_cm = tc.tile_pool(name="psC", bufs=1, space="PSUM")
            psC = psC_cm.__enter__()
            psf_cm = tc.tile_pool(name="psf", bufs=1, space="PSUM")
            psf = psf_cm.__enter__()

            def emit_ffn1(mp):
                for fc in range(32):
                    facc = psf.tile([P, 2, P], f32, tag="facc", bufs=4,
                                    name=f"fa{mp}_{fc}")
                    for kc in range(8):
                        nc.tensor.matmul(
                            facc,
                            lhsT=w1_sb[:, kc, P * fc:P * (fc + 1)],
                            rhs=lnT[:, kc, 2 * mp:2 * mp + 2, :],
                            start=(kc == 0), stop=(kc == 7))
                    nc.scalar.activation(h1T[:, fc, 2 * mp:2 * mp + 2, :],
                                         facc, AF.Relu,
                                         bias=b1T_sb[:, fc:fc + 1])

            emit_norm(2)
            emit_proj(0)
            emit_norm(3)
            emit_proj(1)
            emit_ffn1(0)
            emit_proj(2)
            emit_proj(3)
            emit_ffn1(1)
            psf_cm.__exit__(None, None, None)
            psC_cm.__exit__(None, None, None)
            atn_cm.__exit__(None, None, None)
            wppool_cm.__exit__(None, None, None)

            # ---------------- Phase D: FFN2 -------------------------------
            # w2 slabs on the gpsimd queue: the engine reaches these right
            # after the xr loads, so they overlap attention/proj/FFN1 (the
            # SBUF region reuse fences them behind the atn pool's last use).
            w2pool_cm = tc.tile_pool(name="w2pool", bufs=1, side="right")
            w2pool = w2pool_cm.__enter__()
            w2_sb = w2pool.tile([P, 32, E], fp16)
            for g in range(4):
                nc.gpsimd.dma_start(w2_sb[:, 8 * g:8 * g + 8, :],
                                    w2_d[:, 8 * g:8 * g + 8, :])
            pso_cm = tc.tile_pool(name="pso", bufs=1, space="PSUM")
            pso = pso_cm.__enter__()
            for mi in range(4):
                b, hh = mi // 2, mi % 2
                for ns_i in range(2):
                    ns = slice(ns_i * 512, (ns_i + 1) * 512)
                    oacc = pso.tile([P, 512], f32, tag="oacc", bufs=4,
                                    name=f"oa{mi}_{ns_i}")
                    nc.tensor.matmul(oacc, lhsT=ones_h,
                                     rhs=b2_row[:, ns], start=True,
                                     stop=False)
                    for kcf in range(32):
                        nc.tensor.matmul(
                            oacc,
                            lhsT=h1T[:, kcf, mi, :],
                            rhs=w2_sb[:, kcf, ns],
                            start=False, stop=(kcf == 31))
                    o_half = w2pool.tile([P, 512], f32, tag="o", bufs=4,
                                         name=f"o{mi}_{ns_i}")
                    nc.vector.tensor_copy(o_half, oacc)
                    nc.gpsimd.dma_start(out_d[b, hh, :, ns], o_half)
            pso_cm.__exit__(None, None, None)
            w2pool_cm.__exit__(None, None, None)
            midpool_cm.__exit__(None, None, None)
            w1pool_cm.__exit__(None, None, None)

    nc.compile()
    return nc


def _get_nc():
    if "nc" not in _cached:
        _cached["nc"] = _build()
    return _cached["nc"]


def _make_in_maps(inputs):
    import ml_dtypes
    bf = ml_dtypes.bfloat16
    e4 = ml_dtypes.float8_e4m3
    x = np.ascontiguousarray(np.asarray(inputs["x"], dtype=np.float32))
    w_qkv = np.asarray(inputs["w_qkv"], dtype=np.float32)
    b_qkv = np.asarray(inputs["b_qkv"], dtype=np.float32)
    w_proj = np.asarray(inputs["w_proj"], dtype=np.float32)
    b_proj = np.asarray(inputs["b_proj"], dtype=np.float32)
    ln_g = np.asarray(inputs["ln_g"], dtype=np.float32)
    ln_b = np.asarray(inputs["ln_b"], dtype=np.float32)
    w1 = np.asarray(inputs["w1"], dtype=np.float32)
    b1 = np.asarray(inputs["b1"], dtype=np.float32)
    w2 = np.asarray(inputs["w2"], dtype=np.float32)
    b2 = np.asarray(inputs["b2"], dtype=np.float32)

    w1e = ln_g[:, None] * w1                     # [E, FF]
    b1e = b1 + ln_b @ w1                         # [FF]

    wq_h = np.ascontiguousarray(
        w_qkv.reshape(4, 2, P, ROW).transpose(2, 0, 1, 3)).astype(e4)
    wp_h = np.ascontiguousarray(
        w_proj.reshape(4, 2, P, E).transpose(2, 0, 1, 3)).astype(e4)
    w1_h = np.ascontiguousarray(
        w1e.reshape(8, P, FF).transpose(1, 0, 2)).astype(np.float16)
    w2_h = np.ascontiguousarray(
        w2.reshape(32, P, E).transpose(1, 0, 2)).astype(np.float16)
    b1T_h = np.ascontiguousarray(b1e.reshape(32, P).T).astype(np.float32)
    bq_h = b_qkv.reshape(1, ROW).astype(bf)
    b2_h = b2.reshape(1, E).astype(np.float16)
    triu_h = np.triu(np.ones((P, P))).astype(e4)
    ones_h = np.ones((1, P), np.float32)

    in_maps = []
    slots_l = [(b, t) for b in range(B) for t in range(3)]
    for c in range(NCORES):
        xq_full = np.zeros((P, 8, 544), np.float32)
        offs = np.zeros((1, 4), np.uint32)
        for m, (b, t) in enumerate(slots_l):
            start = (16 * t + 2 * c) * BLK
            T0 = start // ROW
            offs[0, t] = ROW - (start - T0 * ROW)
            n = min(88, S - T0)
            xs = x[b, T0:T0 + n]                 # [n, E]
            xq_full[:, :, 88 * m:88 * m + n] = np.ascontiguousarray(
                xs.T).reshape(8, P, n).transpose(1, 0, 2)
        xqT = xq_full.reshape(P, 4, 2, 544).astype(e4)
        xr = np.zeros((B, 2, P, E), np.float32)
        for hh in range(2):
            h_ = 2 * c + hh
            for b in range(B):
                xr[b, hh] = x[b, P * h_:P * (h_ + 1)] + b_proj
        in_maps.append({
            "xqT": xqT, "xr": xr, "offs": offs,
            "ones": ones_h, "triu": triu_h,
            "wq": wq_h, "bq": bq_h, "wp": wp_h,
            "w1": w1_h, "b1T": b1T_h, "w2": w2_h, "b2": b2_h,
        })
    return in_maps


def _run(inputs, trace=False, trace_cores=None):
    import sys
    if "/opt/trn_rl_repo" not in sys.path:
        sys.path.insert(0, "/opt/trn_rl_repo")
    from concourse.bass_utils import run_bass_kernel_spmd
    nc = _get_nc()
    in_maps = _make_in_maps(inputs)
    kwargs = {}
    if trace:
        kwargs["trace"] = True
        if trace_cores is not None:
            kwargs["trace_cores"] = trace_cores
    res = run_bass_kernel_spmd(nc, in_maps, list(range(NCORES)), **kwargs)
    full = np.zeros((B, S, E), np.float32)
    for c in range(NCORES):
        o = res.results[c]["out"]
        for hh in range(2):
            h_ = 2 * c + hh
            for b in range(B):
                full[b, P * h_:P * (h_ + 1)] = o[b, hh]
    return full, res


def kernel(**inputs) -> np.ndarray:
    import sys
    if "/opt/trn_rl_repo" not in sys.path:
        sys.path.insert(0, "/opt/trn_rl_repo")
    full, _ = _run(inputs)
    return full


# revision 34
# speedup vs baseline: 1.0057x; 1.0057x over previous
# Trainium2 Bass kernel for nn_MultiHeadTransformer (B=2, S=2048, E=1024, H=16, FF=4096).
#
# Sharding: 8-way head/row parallel with zero collectives (the reference's
# "faithful raw view" makes qkv/attention/proj/LN/FFN row-local per core: core c
# computes the qkv rows covering the six flat (type, head) blocks of its two
# heads; the inverse view maps head outputs back to its own 256 token rows).
#
# v5: fp8-e4m3 + DoubleRow on QKV / AV / proj (validated end-to-end rel err
# ~4e-3 vs the 2e-2 gate in a numpy quantization sim; FFN stays fp16 because
# fp8 there alone costs ~4e-2):
#  - QKV: x^T and w_qkv host-packed as [P, 4, 2, *] fp8 kc-pairs; per-mt waves
#    of 6 PSUM accs, 4 DoubleRow passes each; per-g contiguous DRAM slabs so
#    the first matmul starts as soon as slab g0 lands.
#  - scratch (qkv scatter through DRAM) in fp8: half the roundtrip bytes.
#  - q/k staged fp8, PE-transposed fp8, converted to fp8 qT/kT via the
#    existing DVE copy-out; scores run fp8 (same PE rate as bf16) with the
#    two heads' score matmuls emitted interleaved on disjoint PE row halves
#    (rows 0-63 / 64-127) so they execute concurrently.
#  - AV: v staged as [P, 8, 2, 160] fp8 j-pairs (ones row at +64 for the
#    softmax denominator); exp writes fp8 into paired a2 tiles; AV runs one
#    DoubleRow matmul per j-pair, plus a single-j matmul for the causal
#    straddle chunk [s_even, s_odd).
#  - proj: hT scatter written fp8 in kc-pairs, w_proj fp8 [P, 4, 2, E],
#    4 DoubleRow passes per 512-col chunk.
#  - causal masks multiplied on DVE (fp8); gpsimd keeps only scratch writes,
#    xr loads and output writes; output written per 512-col half.
import numpy as np

B, S, E, H, DH, FF = 2, 2048, 1024, 16, 64, 4096
ROW = 3 * E            # 3072 qkv columns
BLK = S * DH           # 131072 elements per (type, head) block
NCORES = 8
P = 128
INV_SCALE = 1.0 / float(np.sqrt(E))

_cached = {}


def _build():
    import concourse.bacc as bacc
    import concourse.bass as bass
    import concourse.mybir as mybir
    import concourse.tile as tile
    from concourse.masks import make_identity

    f32 = mybir.dt.float32
    bf16 = mybir.dt.bfloat16
    fp16 = mybir.dt.float16
    f8 = mybir.dt.float8e4
    u32 = mybir.dt.uint32
    AF = mybir.ActivationFunctionType
    ALU = mybir.AluOpType
    DR = mybir.MatmulPerfMode.DoubleRow

    nc = bacc.Bacc(trn_type="TRN2", target_bir_lowering=False, debug=False,
                   num_devices=NCORES)

    xqT_d = nc.dram_tensor("xqT", [P, 4, 2, 544], f8,
                           kind="ExternalInput").ap()
    wq_d = nc.dram_tensor("wq", [P, 4, 2, ROW], f8, kind="ExternalInput").ap()
    bq_d = nc.dram_tensor("bq", [1, ROW], bf16, kind="ExternalInput").ap()
    wp_d = nc.dram_tensor("wp", [P, 4, 2, E], f8, kind="ExternalInput").ap()
    w1_d = nc.dram_tensor("w1", [P, 8, FF], fp16, kind="ExternalInput").ap()
    b1T_d = nc.dram_tensor("b1T", [P, 32], f32, kind="ExternalInput").ap()
    w2_d = nc.dram_tensor("w2", [P, 32, E], fp16, kind="ExternalInput").ap()
    b2_d = nc.dram_tensor("b2", [1, E], fp16, kind="ExternalInput").ap()
    xr_d = nc.dram_tensor("xr", [B, 2, P, E], f32, kind="ExternalInput").ap()
    offs_d = nc.dram_tensor("offs", [1, 4], u32, kind="ExternalInput").ap()
    triu_d = nc.dram_tensor("triu", [P, P], f8, kind="ExternalInput").ap()
    ones_d = nc.dram_tensor("ones", [1, P], f32, kind="ExternalInput").ap()
    out_d = nc.dram_tensor("out", [B, 2, P, E], f32, kind="ExternalOutput").ap()

    slots = [(b, t) for b in range(B) for t in range(3)]

    with tile.TileContext(nc) as tc:
        with tc.tile_pool(name="singles", bufs=1) as singles, \
             tc.tile_pool(name="dram", bufs=1, space="DRAM") as dram:
            triu = singles.tile([P, P], f8)
            b2_row = singles.tile([1, E], fp16)
            b1T_sb = singles.tile([P, 32], f32)
            ones_f = singles.tile([1, P], f32)
            nc.sync.dma_start(ones_f, ones_d)
            ones_h = singles.tile([1, P], fp16)
            nc.vector.tensor_copy(ones_h, ones_f)
            ident_b = singles.tile([P, P], bf16)
            make_identity(nc, ident_b)
            ident_8 = singles.tile([P, P], f8)
            nc.vector.tensor_copy(ident_8, ident_b)
            ident_h = singles.tile([P, P], fp16)
            make_identity(nc, ident_h)
            eps_t = singles.tile([P, 1], f32)
            nc.vector.memset(eps_t, 1e-5)
            offs_sb = singles.tile([1, 4], u32)
            nc.sync.dma_start(offs_sb, offs_d)
            # offsets are only used by gpsimd (Pool) scratch-write DMAs;
            # loading on all engines costs a ~5us all-engine barrier preamble.
            off_v = [nc.values_load(offs_sb[:, t:t + 1],
                                    engines=[mybir.EngineType.Pool],
                                    min_val=0, max_val=ROW,
                                    skip_runtime_bounds_check=True)
                     for t in range(3)]

            SCR88 = ROW + 88 * ROW
            scr = [[dram.tile([SCR88], f8, tag=f"scr{b}{t}",
                              name=f"scr{b}_{t}") for t in range(3)]
                   for b in range(B)]

            # PSUM pools are phase-scoped: psA (QKV accs + q/k transposes)
            # -> psB (attention sc+oT, 8 banks) -> psC (proj accs).
            psA_cm = tc.tile_pool(name="psA", bufs=1, space="PSUM")
            psA = psA_cm.__enter__()
            midpool_cm = tc.tile_pool(name="midpool", bufs=1, side="right")
            midpool = midpool_cm.__enter__()
            hT = midpool.tile([P, 4, 4, 2, P], f8)
            lnT = midpool.tile([P, 8, 4, P], fp16)
            h1T = midpool.tile([P, 32, 4, P], fp16)
            atn_cm = tc.tile_pool(name="atn", bufs=1, side="right")
            atn = atn_cm.__enter__()

            qs8, ks8, vv, qT, kT = {}, {}, {}, {}, {}

            def emit_read(b):
                # merged scratch reads: one DMA per tensor (q/k/v), both
                # heads, all 16 token blocks.  Issued on sync.
                qs8[b] = atn.tile([P, 16, P], f8, tag="qstage", bufs=2,
                                  name=f"qs{b}")
                ks8[b] = atn.tile([P, 16, P], f8, tag="kstage", bufs=2,
                                  name=f"ks{b}")
                vv[b] = atn.tile([P, 8, 2, 160], f8, tag="v", bufs=2,
                                 name=f"v{b}")
                nc.vector.memset(vv[b][:, :, :, 64:65], 1.0)
                nc.vector.memset(vv[b][:, :, :, 144:145], 1.0)
                def src_of(t, hh):
                    return (scr[b][t][ROW + hh * BLK:ROW + (hh + 1) * BLK]
                            .rearrange("(i p d) -> p i d", p=P, d=DH))
                # q first (its transposes gate the first scores), then k, v
                for hh in range(2):
                    nc.sync.dma_start(
                        qs8[b][:, :, 64 * hh:64 * hh + 64], src_of(0, hh))
                for hh in range(2):
                    nc.sync.dma_start(
                        ks8[b][:, :, 64 * hh:64 * hh + 64], src_of(1, hh))
                for hh in range(2):
                    vdst = vv[b][:, :, :, 80 * hh:80 * hh + 64]
                    nc.sync.dma_start(
                        vdst.rearrange("p g j d -> p (g j) d"), src_of(2, hh))
                qT[b] = atn.tile([P, S], f8, tag="qT", bufs=2,
                                 name=f"qT{b}")
                kT[b] = atn.tile([P, S], f8, tag="kT", bufs=2,
                                 name=f"kT{b}")

            def emit_tp(b):
                # all q transposes first: the first scores only need qT+kT[0]
                for st, dst in ((qs8[b], qT[b]), (ks8[b], kT[b])):
                    for i in range(16):
                        # fp8 PE transpose writes 2-byte slots: use a
                        # stride-2 output AP (low byte of each pair).
                        t_ps = psA.tile([P, 1024, 2], f8, tag="tp", bufs=2,
                                        name=f"tp{b}_{i}")
                        nc.tensor.transpose(t_ps[:, 0:P, 0], st[:, i, :],
                                            ident_8)
                        nc.vector.tensor_copy(dst[:, P * i:P * (i + 1)],
                                              t_ps[:, 0:P, 0])

            # ---------------- Phase A: QKV fp8 DoubleRow ------------------
            poolA_cm = tc.tile_pool(name="poolA", bufs=1)
            poolA = poolA_cm.__enter__()
            wq_sb = poolA.tile([P, 4, 2, ROW], f8)
            xqT_sb = poolA.tile([P, 4, 2, 544], f8)
            bias_bc = poolA.tile([P, ROW], bf16)
            # per-g slabs, g0 first so matmuls start ~3us in; wq slab split
            # in halves so wave g can begin on its first columns.
            for g in range(4):
                nc.sync.dma_start(xqT_sb[:, g], xqT_d[:, g])
                nc.sync.dma_start(wq_sb[:, g, :, 0:1536],
                                  wq_d[:, g, :, 0:1536])
                nc.sync.dma_start(wq_sb[:, g, :, 1536:ROW],
                                  wq_d[:, g, :, 1536:ROW])
            nc.sync.dma_start(bias_bc, bq_d.to_broadcast([P, ROW]))
            nc.sync.dma_start(triu, triu_d)
            nc.sync.dma_start(b2_row, b2_d)
            nc.sync.dma_start(b1T_sb, b1T_d)

            # 528 packed rows -> 5 M-tiles of <=128; slot m=(b,t) owns
            # global rows [88m, 88m+88).
            yts = {}
            for mt in range(5):
                g0, g1 = 128 * mt, min(128 * mt + 128, 528)
                M = g1 - g0
                y = poolA.tile([P, ROW], f8, tag="y", bufs=2,
                               name=f"y{mt}")
                yts[mt] = y
                accs = [psA.tile([P, 512], f32, tag="acc", bufs=6,
                                 name=f"qa{mt}_{n6}") for n6 in range(6)]
                for g in range(4):
                    for n6 in range(6):
                        ns = slice(n6 * 512, (n6 + 1) * 512)
                        nc.tensor.matmul(accs[n6][:M, :],
                                         lhsT=xqT_sb[:, g, :, g0:g1],
                                         rhs=wq_sb[:, g, :, ns],
                                         start=(g == 0), stop=(g == 3),
                                         perf_mode=DR)
                for n6 in range(6):
                    ns = slice(n6 * 512, (n6 + 1) * 512)
                    nc.vector.tensor_add(y[:M, ns], accs[n6][:M, :],
                                         bias_bc[:M, ns])
                for m, (b, t) in enumerate(slots):
                    s0, s1 = 88 * m, 88 * m + 88
                    if mt != (s1 - 1) // 128:
                        continue
                    # rows of this slot, split by containing tile
                    spans = []
                    for tt in (mt - 1, mt):
                        if tt < 0:
                            continue
                        lo = max(s0, 128 * tt)
                        hi = min(s1, 128 * tt + 128)
                        if lo < hi:
                            spans.append((tt, lo, hi))
                    dst = scr[b][t][bass.ds(off_v[t], 88 * ROW)]
                    dst_r = dst.rearrange("(r c) -> r c", c=ROW)
                    for tt, lo, hi in spans:
                        nc.gpsimd.dma_start(
                            dst_r[lo - s0:hi - s0, :],
                            yts[tt][lo - 128 * tt:hi - 128 * tt, :])
                    if m == 2:
                        emit_read(0)
            poolA_cm.__exit__(None, None, None)

            w1pool_cm = tc.tile_pool(name="w1pool", bufs=1)
            w1pool = w1pool_cm.__enter__()
            w1_sb = w1pool.tile([P, 8, FF], fp16)
            wppool_cm = tc.tile_pool(name="wppool", bufs=1)
            wppool = wppool_cm.__enter__()
            wp_sb = wppool.tile([P, 4, 2, E], f8)
            emit_read(1)
            emit_tp(0)
            emit_tp(1)
            nc.sync.dma_start(wp_sb, wp_d)
            nc.sync.dma_start(w1_sb[:, 0:2, :], w1_d[:, 0:2, :])
            # residual inputs preloaded on the gpsimd queue (idle here) so
            # they are never stuck behind the w2 slabs later.
            xr_sbs = {}
            for mi in range(4):
                xr_sbs[mi] = atn.tile([P, E], f32, tag="xr", bufs=4,
                                      name=f"xr{mi}")
                nc.gpsimd.dma_start(xr_sbs[mi], xr_d[mi // 2, mi % 2])
            psA_cm.__exit__(None, None, None)
            psB_cm = tc.tile_pool(name="psB", bufs=1, space="PSUM")
            psB = psB_cm.__enter__()

            # ------------- Phase B+C: attention / proj / LN ---------------
            # Query-quarter blocks (512 queries): both heads' scores share
            # one 2-bank PSUM tile, so exp runs ONCE per j for both heads
            # (halves ACT instruction overhead); sc bufs=3 gives a 2-j
            # lookahead so ACT never starves; AV per j-pair via DoubleRow.
            oT_sbs, dnrows, wraps = {}, {}, {}

            def emit_pair(b):
                for hh in range(2):
                    mi = 2 * b + hh
                    oT_sbs[mi] = atn.tile([64, S], bf16, tag="oTsb",
                                          bufs=2, name=f"oTsb{mi}")
                    dnrows[mi] = atn.tile([1, S], bf16, tag="dnrow",
                                          bufs=2, name=f"dn{mi}")
                    wraps[mi] = atn.tile([P, 16], bf16, tag="wrap",
                                         bufs=2, name=f"wrap{mi}")

                for Qq in range(4):
                    Q0 = 512 * Qq
                    jmax = 4 * (Qq + 1)
                    oT = {hh: psB.tile([65, 512], f32, tag="oT", bufs=2,
                                       name=f"oT{b}{hh}_{Qq}")
                          for hh in range(2)}

                    def emit_scores(j):
                        s = max(128 * j - Q0, 0)
                        t_sc = psB.tile([P, 2, 512], f32, tag="sc",
                                        bufs=3, name=f"sc{b}_{Qq}_{j}")
                        for hh in range(2):
                            hp = slice(64 * hh, 64 * hh + 64)
                            nc.tensor.matmul(
                                t_sc[:, hh, s:512],
                                lhsT=kT[b][hp, P * j:P * (j + 1)],
                                rhs=qT[b][hp, Q0 + s:Q0 + 512],
                                start=True, stop=True)
                        return t_sc

                    sc = emit_scores(0)
                    a2 = None
                    for j in range(jmax):
                        gp, jo = j // 2, j % 2
                        s = max(128 * j - Q0, 0)
                        if jo == 0:
                            a2 = atn.tile([P, 2, 2, 512], f8, tag="a",
                                          bufs=4, name=f"a{b}_{Qq}_{gp}")
                        nc.scalar.activation(a2[:, jo, :, s:512],
                                             sc[:, :, s:512],
                                             AF.Exp,
                                             scale=float(INV_SCALE))
                        if 128 * j >= Q0:
                            for hh in range(2):
                                nc.vector.tensor_mul(
                                    a2[:, jo, hh, s:s + P],
                                    a2[:, jo, hh, s:s + P], triu)
                        if j + 1 < jmax:
                            sc = emit_scores(j + 1)
                        if jo == 1:
                            s0 = max(128 * (j - 1) - Q0, 0)
                            s1 = s
                            for hh in range(2):
                                if s1 > s0:
                                    nc.tensor.matmul(
                                        oT[hh][:, s0:s1],
                                        lhsT=vv[b][:, gp, 0,
                                                   80 * hh:80 * hh + 65],
                                        rhs=a2[:, 0, hh, s0:s1],
                                        start=(gp == 0), stop=False)
                                nc.tensor.matmul(
                                    oT[hh][:, s1:512],
                                    lhsT=vv[b][:, gp, :,
                                               80 * hh:80 * hh + 65],
                                    rhs=a2[:, :, hh, s1:512],
                                    start=(gp == 0),
                                    stop=(gp == 2 * Qq + 1),
                                    perf_mode=DR)
                    for hh in range(2):
                        mi = 2 * b + hh
                        nc.vector.tensor_copy(
                            oT_sbs[mi][:, Q0:Q0 + 512], oT[hh][0:64, :])
                        nc.vector.tensor_copy(
                            dnrows[mi][:, Q0:Q0 + 512], oT[hh][64:65, :])
                        dnd_fq = dram.tile([1, 512], bf16, tag="dndf",
                                           bufs=8, name=f"dndf{mi}_{Qq}")
                        nc.sync.dma_start(dnd_fq,
                                          dnrows[mi][:, Q0:Q0 + 512])
                        nc.sync.dma_start(
                            wraps[mi][32 * Qq:32 * Qq + 32, :],
                            dnd_fq[0, :].rearrange("(p f) -> p f", f=16))

            def emit_norm(mi):
                b, hh = mi // 2, mi % 2
                oT_sb = oT_sbs.pop(mi)
                dnrows.pop(mi)
                wrap = wraps.pop(mi)
                wrap_b = atn.tile([P, 16], bf16, tag="wrapb", bufs=2,
                                  name=f"wrapb{mi}")
                with nc.allow_low_precision(reason="softmax denom recip"):
                    nc.vector.reciprocal(wrap_b, wrap)
                dnd_b = dram.tile([1, S], bf16, tag="dndb", bufs=2,
                                  name=f"dndb{mi}")
                nc.sync.dma_start(
                    dnd_b[0, :].rearrange("(p f) -> p f", f=16), wrap_b)
                rrep = atn.tile([64, S], bf16, tag="rrep", bufs=2,
                                name=f"rrep{mi}")
                nc.sync.dma_start(rrep, dnd_b.to_broadcast([64, S]))
                # normalization folded into the proj-lhsT scatter (fp8 out)
                oT_r = oT_sb.rearrange("d (t a) -> d a t", a=16)
                rr_r = rrep.rearrange("d (t a) -> d a t", a=16)
                # mi0/1: half the muls on gpsimd so the DVE FIFO block in
                # front of b1's masks halves; mi2/3 post-attention stay DVE
                # (keeps the gpsimd queue clear to issue the w2 slabs).
                for kc in range(8):
                    for ah in range(2):
                        eng = (nc.gpsimd if (mi < 2 and ah == 1)
                               else nc.vector)
                        eng.tensor_mul(
                            hT[64 * ah:64 * ah + 64, mi, kc // 2, kc % 2, :],
                            oT_r[:, 2 * kc + ah, :],
                            rr_r[:, 2 * kc + ah, :])

            def emit_proj(mi):
                b, hh = mi // 2, mi % 2
                xr_sb = xr_sbs[mi]
                r_sb = atn.tile([P, E], f32, tag="r", bufs=2,
                                name=f"r{mi}")
                for ns_i in range(2):
                    ns = slice(ns_i * 512, (ns_i + 1) * 512)
                    pacc = psC.tile([P, 512], f32, tag="pacc", bufs=2,
                                    name=f"pa{mi}_{ns_i}")
                    for g in range(4):
                        nc.tensor.matmul(pacc, lhsT=hT[:, mi, g, :, :],
                                         rhs=wp_sb[:, g, :, ns],
                                         start=(g == 0),
                                         stop=(g == 3),
                                         perf_mode=DR)
                    nc.vector.tensor_add(r_sb[:, ns], pacc,
                                         xr_sb[:, ns])
                stats = atn.tile([P, 2, 6], f32, tag="stats", bufs=2,
                                 name=f"st{mi}")
                for sg in range(2):
                    nc.vector.bn_stats(stats[:, sg, :],
                                       r_sb[:, sg * 512:(sg + 1) * 512])
                mv = atn.tile([P, 2], f32, tag="mv", bufs=2,
                              name=f"mv{mi}")
                nc.vector.bn_aggr(mv, stats)
                nc.scalar.activation(mv[:, 1:2], mv[:, 1:2], AF.Sqrt,
                                     bias=eps_t, scale=1.0)
                nc.vector.reciprocal(mv[:, 1:2], mv[:, 1:2])
                ln_m = atn.tile([P, E], fp16, tag="ln", bufs=2,
                                name=f"ln{mi}")
                nc.vector.tensor_scalar(ln_m, r_sb, mv[:, 0:1],
                                        mv[:, 1:2], ALU.subtract,
                                        ALU.mult)
                for kc in range(8):
                    t_ps = psC.tile([P, 1024], fp16, tag="tpl", bufs=2,
                                    name=f"tpl{mi}_{kc}")
                    nc.tensor.transpose(t_ps[:, 0:P],
                                        ln_m[:, P * kc:P * (kc + 1)],
                                        ident_h)
                    nc.vector.tensor_copy(lnT[:, kc, mi, :], t_ps[:, 0:P])

            emit_pair(0)
            nc.sync.dma_start(w1_sb[:, 2:5, :], w1_d[:, 2:5, :])
            emit_norm(0)
            emit_norm(1)
            emit_pair(1)
            nc.sync.dma_start(w1_sb[:, 5:8, :], w1_d[:, 5:8, :])
            psB_cm.__exit__(None, None, None)
            # proj (4 banks) + FFN1 accs (2 banks) coexist: FFN1 is split by
            # mi-pair (N=256) and interleaved with the proj emissions so the
            # PE never idles >3.4us here (HAM stays at full clock).
            psC_cm = tc.tile_pool(name="psC", bufs=1, space="PSUM")
            psC = psC_cm.__enter__()
            psf_cm = tc.tile_pool(name="psf", bufs=1, space="PSUM")
            psf = psf_cm.__enter__()

            def emit_ffn1(mp):
                for fc in range(32):
                    facc = psf.tile([P, 2, P], f32, tag="facc", bufs=4,
                                    name=f"fa{mp}_{fc}")
                    for kc in range(8):
                        nc.tensor.matmul(
                            facc,
                            lhsT=w1_sb[:, kc, P * fc:P * (fc + 1)],
                            rhs=lnT[:, kc, 2 * mp:2 * mp + 2, :],
                            start=(kc == 0), stop=(kc == 7))
                    nc.scalar.activation(h1T[:, fc, 2 * mp:2 * mp + 2, :],
                                         facc, AF.Relu,
                                         bias=b1T_sb[:, fc:fc + 1])

            emit_norm(2)
            emit_proj(0)
            emit_norm(3)
            emit_proj(1)
            emit_ffn1(0)
            emit_proj(2)
            emit_proj(3)
            emit_ffn1(1)
            psf_cm.__exit__(None, None, None)
            psC_cm.__exit__(None, None, None)
            atn_cm.__exit__(None, None, None)
            wppool_cm.__exit__(None, None, None)

            # ---------------- Phase D: FFN2 -------------------------------
            # w2 slabs on the gpsimd queue: the engine reaches these right
            # after the xr loads, so they overlap attention/proj/FFN1 (the
            # SBUF region reuse fences them behind the atn pool's last use).
            w2pool_cm = tc.tile_pool(name="w2pool", bufs=1, side="right")
            w2pool = w2pool_cm.__enter__()
            w2_sb = w2pool.tile([P, 32, E], fp16)
            for g in range(4):
                nc.gpsimd.dma_start(w2_sb[:, 8 * g:8 * g + 8, :],
                                    w2_d[:, 8 * g:8 * g + 8, :])
            pso_cm = tc.tile_pool(name="pso", bufs=1, space="PSUM")
            pso = pso_cm.__enter__()
            for mi in range(4):
                b, hh = mi // 2, mi % 2
                for ns_i in range(2):
                    ns = slice(ns_i * 512, (ns_i + 1) * 512)
                    oacc = pso.tile([P, 512], f32, tag="oacc", bufs=4,
                                    name=f"oa{mi}_{ns_i}")
                    nc.tensor.matmul(oacc, lhsT=ones_h,
                                     rhs=b2_row[:, ns], start=True,
                                     stop=False)
                    for kcf in range(32):
                        nc.tensor.matmul(
                            oacc,
                            lhsT=h1T[:, kcf, mi, :],
                            rhs=w2_sb[:, kcf, ns],
                            start=False, stop=(kcf == 31))
                    o_half = w2pool.tile([P, 512], f32, tag="o", bufs=4,
                                         name=f"o{mi}_{ns_i}")
                    nc.vector.tensor_copy(o_half, oacc)
                    nc.gpsimd.dma_start(out_d[b, hh, :, ns], o_half)
            pso_cm.__exit__(None, None, None)
            w2pool_cm.__exit__(None, None, None)
            midpool_cm.__exit__(None, None, None)
            w1pool_cm.__exit__(None, None, None)

    nc.compile()
    return nc


def _get_nc():
    if "nc" not in _cached:
        _cached["nc"] = _build()
    return _cached["nc"]


def _make_in_maps(inputs):
    import ml_dtypes
    bf = ml_dtypes.bfloat16
    e4 = ml_dtypes.float8_e4m3
    x = np.ascontiguousarray(np.asarray(inputs["x"], dtype=np.float32))
    w_qkv = np.asarray(inputs["w_qkv"], dtype=np.float32)
    b_qkv = np.asarray(inputs["b_qkv"], dtype=np.float32)
    w_proj = np.asarray(inputs["w_proj"], dtype=np.float32)
    b_proj = np.asarray(inputs["b_proj"], dtype=np.float32)
    ln_g = np.asarray(inputs["ln_g"], dtype=np.float32)
    ln_b = np.asarray(inputs["ln_b"], dtype=np.float32)
    w1 = np.asarray(inputs["w1"], dtype=np.float32)
    b1 = np.asarray(inputs["b1"], dtype=np.float32)
    w2 = np.asarray(inputs["w2"], dtype=np.float32)
    b2 = np.asarray(inputs["b2"], dtype=np.float32)

    w1e = ln_g[:, None] * w1                     # [E, FF]
    b1e = b1 + ln_b @ w1                         # [FF]

    wq_h = np.ascontiguousarray(
        w_qkv.reshape(4, 2, P, ROW).transpose(2, 0, 1, 3)).astype(e4)
    wp_h = np.ascontiguousarray(
        w_proj.reshape(4, 2, P, E).transpose(2, 0, 1, 3)).astype(e4)
    w1_h = np.ascontiguousarray(
        w1e.reshape(8, P, FF).transpose(1, 0, 2)).astype(np.float16)
    w2_h = np.ascontiguousarray(
        w2.reshape(32, P, E).transpose(1, 0, 2)).astype(np.float16)
    b1T_h = np.ascontiguousarray(b1e.reshape(32, P).T).astype(np.float32)
    bq_h = b_qkv.reshape(1, ROW).astype(bf)
    b2_h = b2.reshape(1, E).astype(np.float16)
    triu_h = np.triu(np.ones((P, P))).astype(e4)
    ones_h = np.ones((1, P), np.float32)

    in_maps = []
    slots_l = [(b, t) for b in range(B) for t in range(3)]
    for c in range(NCORES):
        xq_full = np.zeros((P, 8, 544), np.float32)
        offs = np.zeros((1, 4), np.uint32)
        for m, (b, t) in enumerate(slots_l):
            start = (16 * t + 2 * c) * BLK
            T0 = start // ROW
            offs[0, t] = ROW - (start - T0 * ROW)
            n = min(88, S - T0)
            xs = x[b, T0:T0 + n]                 # [n, E]
            xq_full[:, :, 88 * m:88 * m + n] = np.ascontiguousarray(
                xs.T).reshape(8, P, n).transpose(1, 0, 2)
        xqT = xq_full.reshape(P, 4, 2, 544).astype(e4)
        xr = np.zeros((B, 2, P, E), np.float32)
        for hh in range(2):
            h_ = 2 * c + hh
            for b in range(B):
                xr[b, hh] = x[b, P * h_:P * (h_ + 1)] + b_proj
        in_maps.append({
            "xqT": xqT, "xr": xr, "offs": offs,
            "ones": ones_h, "triu": triu_h,
            "wq": wq_h, "bq": bq_h, "wp": wp_h,
            "w1": w1_h, "b1T": b1T_h, "w2": w2_h, "b2": b2_h,
        })
    return in_maps


def _run(inputs, trace=False, trace_cores=None):
    import sys
    if "/opt/trn_rl_repo" not in sys.path:
        sys.path.insert(0, "/opt/trn_rl_repo")
    from concourse.bass_utils import run_bass_kernel_spmd
    nc = _get_nc()
    in_maps = _make_in_maps(inputs)
    kwargs = {}
    if trace:
        kwargs["trace"] = True
        if trace_cores is not None:
            kwargs["trace_cores"] = trace_cores
    res = run_bass_kernel_spmd(nc, in_maps, list(range(NCORES)), **kwargs)
    full = np.zeros((B, S, E), np.float32)
    for c in range(NCORES):
        o = res.results[c]["out"]
        for hh in range(2):
            h_ = 2 * c + hh
            for b in range(B):
                full[b, P * h_:P * (h_ + 1)] = o[b, hh]
    return full, res


def kernel(**inputs) -> np.ndarray:
    import sys
    if "/opt/trn_rl_repo" not in sys.path:
        sys.path.insert(0, "/opt/trn_rl_repo")
    full, _ = _run(inputs)
    return full


# revision 35
# speedup vs baseline: 1.0136x; 1.0079x over previous
# Trainium2 Bass kernel for nn_MultiHeadTransformer (B=2, S=2048, E=1024, H=16, FF=4096).
#
# Sharding: 8-way head/row parallel with zero collectives (the reference's
# "faithful raw view" makes qkv/attention/proj/LN/FFN row-local per core: core c
# computes the qkv rows covering the six flat (type, head) blocks of its two
# heads; the inverse view maps head outputs back to its own 256 token rows).
#
# v5: fp8-e4m3 + DoubleRow on QKV / AV / proj (validated end-to-end rel err
# ~4e-3 vs the 2e-2 gate in a numpy quantization sim; FFN stays fp16 because
# fp8 there alone costs ~4e-2):
#  - QKV: x^T and w_qkv host-packed as [P, 4, 2, *] fp8 kc-pairs; per-mt waves
#    of 6 PSUM accs, 4 DoubleRow passes each; per-g contiguous DRAM slabs so
#    the first matmul starts as soon as slab g0 lands.
#  - scratch (qkv scatter through DRAM) in fp8: half the roundtrip bytes.
#  - q/k staged fp8, PE-transposed fp8, converted to fp8 qT/kT via the
#    existing DVE copy-out; scores run fp8 (same PE rate as bf16) with the
#    two heads' score matmuls emitted interleaved on disjoint PE row halves
#    (rows 0-63 / 64-127) so they execute concurrently.
#  - AV: v staged as [P, 8, 2, 160] fp8 j-pairs (ones row at +64 for the
#    softmax denominator); exp writes fp8 into paired a2 tiles; AV runs one
#    DoubleRow matmul per j-pair, plus a single-j matmul for the causal
#    straddle chunk [s_even, s_odd).
#  - proj: hT scatter written fp8 in kc-pairs, w_proj fp8 [P, 4, 2, E],
#    4 DoubleRow passes per 512-col chunk.
#  - causal masks multiplied on DVE (fp8); gpsimd keeps only scratch writes,
#    xr loads and output writes; output written per 512-col half.
import numpy as np

B, S, E, H, DH, FF = 2, 2048, 1024, 16, 64, 4096
ROW = 3 * E            # 3072 qkv columns
BLK = S * DH           # 131072 elements per (type, head) block
NCORES = 8
P = 128
INV_SCALE = 1.0 / float(np.sqrt(E))

_cached = {}


def _build():
    import concourse.bacc as bacc
    import concourse.bass as bass
    import concourse.mybir as mybir
    import concourse.tile as tile
    from concourse.masks import make_identity

    f32 = mybir.dt.float32
    bf16 = mybir.dt.bfloat16
    fp16 = mybir.dt.float16
    f8 = mybir.dt.float8e4
    u32 = mybir.dt.uint32
    AF = mybir.ActivationFunctionType
    ALU = mybir.AluOpType
    DR = mybir.MatmulPerfMode.DoubleRow

    nc = bacc.Bacc(trn_type="TRN2", target_bir_lowering=False, debug=False,
                   num_devices=NCORES)

    xqT_d = nc.dram_tensor("xqT", [P, 4, 2, 544], f8,
                           kind="ExternalInput").ap()
    wq_d = nc.dram_tensor("wq", [P, 4, 2, ROW], f8, kind="ExternalInput").ap()
    bq_d = nc.dram_tensor("bq", [1, ROW], bf16, kind="ExternalInput").ap()
    wp_d = nc.dram_tensor("wp", [P, 4, 2, E], f8, kind="ExternalInput").ap()
    w1_d = nc.dram_tensor("w1", [P, 8, FF], fp16, kind="ExternalInput").ap()
    b1T_d = nc.dram_tensor("b1T", [P, 32], f32, kind="ExternalInput").ap()
    w2_d = nc.dram_tensor("w2", [P, 32, E], fp16, kind="ExternalInput").ap()
    b2_d = nc.dram_tensor("b2", [1, E], fp16, kind="ExternalInput").ap()
    xr_d = nc.dram_tensor("xr", [B, 2, P, E], f32, kind="ExternalInput").ap()
    offs_d = nc.dram_tensor("offs", [1, 4], u32, kind="ExternalInput").ap()
    triu_d = nc.dram_tensor("triu", [P, P], f8, kind="ExternalInput").ap()
    ones_d = nc.dram_tensor("ones", [1, P], f32, kind="ExternalInput").ap()
    out_d = nc.dram_tensor("out", [B, 2, P, E], f32, kind="ExternalOutput").ap()

    slots = [(b, t) for b in range(B) for t in range(3)]

    with tile.TileContext(nc) as tc:
        with tc.tile_pool(name="singles", bufs=1) as singles, \
             tc.tile_pool(name="dram", bufs=1, space="DRAM") as dram:
            triu = singles.tile([P, P], f8)
            b2_row = singles.tile([1, E], fp16)
            b1T_sb = singles.tile([P, 32], f32)
            ones_f = singles.tile([1, P], f32)
            nc.sync.dma_start(ones_f, ones_d)
            ones_h = singles.tile([1, P], fp16)
            nc.vector.tensor_copy(ones_h, ones_f)
            ident_b = singles.tile([P, P], bf16)
            make_identity(nc, ident_b)
            ident_8 = singles.tile([P, P], f8)
            nc.vector.tensor_copy(ident_8, ident_b)
            ident_h = singles.tile([P, P], fp16)
            make_identity(nc, ident_h)
            eps_t = singles.tile([P, 1], f32)
            nc.vector.memset(eps_t, 1e-5)
            offs_sb = singles.tile([1, 4], u32)
            nc.sync.dma_start(offs_sb, offs_d)
            # offsets are only used by gpsimd (Pool) scratch-write DMAs;
            # loading on all engines costs a ~5us all-engine barrier preamble.
            off_v = [nc.values_load(offs_sb[:, t:t + 1],
                                    engines=[mybir.EngineType.Pool],
                                    min_val=0, max_val=ROW,
                                    skip_runtime_bounds_check=True)
                     for t in range(3)]

            SCR88 = ROW + 88 * ROW
            scr = [[dram.tile([SCR88], f8, tag=f"scr{b}{t}",
                              name=f"scr{b}_{t}") for t in range(3)]
                   for b in range(B)]

            # PSUM pools are phase-scoped: psA (QKV accs + q/k transposes)
            # -> psB (attention sc+oT, 8 banks) -> psC (proj accs).
            psA_cm = tc.tile_pool(name="psA", bufs=1, space="PSUM")
            psA = psA_cm.__enter__()
            midpool_cm = tc.tile_pool(name="midpool", bufs=1, side="right")
            midpool = midpool_cm.__enter__()
            hT = midpool.tile([P, 4, 4, 2, P], f8)
            lnT = midpool.tile([P, 8, 4, P], fp16)
            h1T = midpool.tile([P, 32, 4, P], fp16)
            atn_cm = tc.tile_pool(name="atn", bufs=1, side="right")
            atn = atn_cm.__enter__()

            qs8, ks8, vv, qT, kT = {}, {}, {}, {}, {}

            def emit_read(b):
                # merged scratch reads: one DMA per tensor (q/k/v), both
                # heads, all 16 token blocks.  Issued on sync.
                qs8[b] = atn.tile([P, 16, P], f8, tag="qstage", bufs=2,
                                  name=f"qs{b}")
                ks8[b] = atn.tile([P, 16, P], f8, tag="kstage", bufs=2,
                                  name=f"ks{b}")
                vv[b] = atn.tile([P, 8, 2, 160], f8, tag="v", bufs=2,
                                 name=f"v{b}")
                nc.vector.memset(vv[b][:, :, :, 64:65], 1.0)
                nc.vector.memset(vv[b][:, :, :, 144:145], 1.0)
                def src_of(t, hh):
                    return (scr[b][t][ROW + hh * BLK:ROW + (hh + 1) * BLK]
                            .rearrange("(i p d) -> p i d", p=P, d=DH))
                # q first (its transposes gate the first scores), then k, v
                for hh in range(2):
                    nc.sync.dma_start(
                        qs8[b][:, :, 64 * hh:64 * hh + 64], src_of(0, hh))
                for hh in range(2):
                    nc.sync.dma_start(
                        ks8[b][:, :, 64 * hh:64 * hh + 64], src_of(1, hh))
                for hh in range(2):
                    vdst = vv[b][:, :, :, 80 * hh:80 * hh + 64]
                    nc.sync.dma_start(
                        vdst.rearrange("p g j d -> p (g j) d"), src_of(2, hh))
                qT[b] = atn.tile([P, S], f8, tag="qT", bufs=2,
                                 name=f"qT{b}")
                kT[b] = atn.tile([P, S], f8, tag="kT", bufs=2,
                                 name=f"kT{b}")

            def emit_tp(b):
                # all q transposes first: the first scores only need qT+kT[0]
                for st, dst in ((qs8[b], qT[b]), (ks8[b], kT[b])):
                    for i in range(16):
                        # fp8 PE transpose writes 2-byte slots: use a
                        # stride-2 output AP (low byte of each pair).
                        t_ps = psA.tile([P, 1024, 2], f8, tag="tp", bufs=2,
                                        name=f"tp{b}_{i}")
                        nc.tensor.transpose(t_ps[:, 0:P, 0], st[:, i, :],
                                            ident_8)
                        nc.vector.tensor_copy(dst[:, P * i:P * (i + 1)],
                                              t_ps[:, 0:P, 0])

            # ---------------- Phase A: QKV fp8 DoubleRow ------------------
            poolA_cm = tc.tile_pool(name="poolA", bufs=1)
            poolA = poolA_cm.__enter__()
            wq_sb = poolA.tile([P, 4, 2, ROW], f8)
            xqT_sb = poolA.tile([P, 4, 2, 544], f8)
            bias_bc = poolA.tile([P, ROW], bf16)
            # per-g slabs, g0 first so matmuls start ~3us in; wq slab split
            # in halves so wave g can begin on its first columns.
            for g in range(4):
                nc.sync.dma_start(xqT_sb[:, g], xqT_d[:, g])
                nc.sync.dma_start(wq_sb[:, g, :, 0:1536],
                                  wq_d[:, g, :, 0:1536])
                nc.sync.dma_start(wq_sb[:, g, :, 1536:ROW],
                                  wq_d[:, g, :, 1536:ROW])
            nc.sync.dma_start(bias_bc, bq_d.to_broadcast([P, ROW]))
            nc.sync.dma_start(triu, triu_d)
            nc.sync.dma_start(b2_row, b2_d)
            nc.sync.dma_start(b1T_sb, b1T_d)

            # 528 packed rows -> 5 M-tiles of <=128; slot m=(b,t) owns
            # global rows [88m, 88m+88).
            yts = {}
            for mt in range(5):
                g0, g1 = 128 * mt, min(128 * mt + 128, 528)
                M = g1 - g0
                y = poolA.tile([P, ROW], f8, tag="y", bufs=2,
                               name=f"y{mt}")
                yts[mt] = y
                accs = [psA.tile([P, 512], f32, tag="acc", bufs=6,
                                 name=f"qa{mt}_{n6}") for n6 in range(6)]
                for g in range(4):
                    for n6 in range(6):
                        ns = slice(n6 * 512, (n6 + 1) * 512)
                        nc.tensor.matmul(accs[n6][:M, :],
                                         lhsT=xqT_sb[:, g, :, g0:g1],
                                         rhs=wq_sb[:, g, :, ns],
                                         start=(g == 0), stop=(g == 3),
                                         perf_mode=DR)
                for n6 in range(6):
                    ns = slice(n6 * 512, (n6 + 1) * 512)
                    nc.vector.tensor_add(y[:M, ns], accs[n6][:M, :],
                                         bias_bc[:M, ns])
                for m, (b, t) in enumerate(slots):
                    s0, s1 = 88 * m, 88 * m + 88
                    if mt != (s1 - 1) // 128:
                        continue
                    # rows of this slot, split by containing tile
                    spans = []
                    for tt in (mt - 1, mt):
                        if tt < 0:
                            continue
                        lo = max(s0, 128 * tt)
                        hi = min(s1, 128 * tt + 128)
                        if lo < hi:
                            spans.append((tt, lo, hi))
                    dst = scr[b][t][bass.ds(off_v[t], 88 * ROW)]
                    dst_r = dst.rearrange("(r c) -> r c", c=ROW)
                    for tt, lo, hi in spans:
                        nc.gpsimd.dma_start(
                            dst_r[lo - s0:hi - s0, :],
                            yts[tt][lo - 128 * tt:hi - 128 * tt, :])
                    if m == 2:
                        emit_read(0)
            poolA_cm.__exit__(None, None, None)

            w1pool_cm = tc.tile_pool(name="w1pool", bufs=1)
            w1pool = w1pool_cm.__enter__()
            w1_sb = w1pool.tile([P, 8, FF], fp16)
            wppool_cm = tc.tile_pool(name="wppool", bufs=1)
            wppool = wppool_cm.__enter__()
            wp_sb = wppool.tile([P, 4, 2, E], f8)
            emit_read(1)
            emit_tp(0)
            emit_tp(1)
            nc.sync.dma_start(wp_sb, wp_d)
            nc.sync.dma_start(w1_sb[:, 0:2, :], w1_d[:, 0:2, :])
            # residual inputs preloaded on the gpsimd queue (idle here) so
            # they are never stuck behind the w2 slabs later.
            xr_sbs = {}
            for mi in range(4):
                xr_sbs[mi] = atn.tile([P, E], f32, tag="xr", bufs=4,
                                      name=f"xr{mi}")
                nc.gpsimd.dma_start(xr_sbs[mi], xr_d[mi // 2, mi % 2])
            psA_cm.__exit__(None, None, None)
            psB_cm = tc.tile_pool(name="psB", bufs=1, space="PSUM")
            psB = psB_cm.__enter__()

            # ------------- Phase B+C: attention / proj / LN ---------------
            # Query-quarter blocks (512 queries): both heads' scores share
            # one 2-bank PSUM tile, so exp runs ONCE per j for both heads
            # (halves ACT instruction overhead); sc bufs=3 gives a 2-j
            # lookahead so ACT never starves; AV per j-pair via DoubleRow.
            oT_sbs, dnrows = {}, {}

            def emit_pair(b):
                for hh in range(2):
                    mi = 2 * b + hh
                    oT_sbs[mi] = atn.tile([64, S], bf16, tag="oTsb",
                                          bufs=2, name=f"oTsb{mi}")
                    dnrows[mi] = atn.tile([1, S], bf16, tag="dnrow",
                                          bufs=2, name=f"dn{mi}")

                for Qq in range(4):
                    Q0 = 512 * Qq
                    jmax = 4 * (Qq + 1)
                    oT = {hh: psB.tile([65, 512], f32, tag="oT", bufs=2,
                                       name=f"oT{b}{hh}_{Qq}")
                          for hh in range(2)}

                    def emit_scores(j):
                        s = max(128 * j - Q0, 0)
                        t_sc = psB.tile([P, 2, 512], f32, tag="sc",
                                        bufs=3, name=f"sc{b}_{Qq}_{j}")
                        for hh in range(2):
                            hp = slice(64 * hh, 64 * hh + 64)
                            nc.tensor.matmul(
                                t_sc[:, hh, s:512],
                                lhsT=kT[b][hp, P * j:P * (j + 1)],
                                rhs=qT[b][hp, Q0 + s:Q0 + 512],
                                start=True, stop=True)
                        return t_sc

                    sc = emit_scores(0)
                    a2 = None
                    for j in range(jmax):
                        gp, jo = j // 2, j % 2
                        s = max(128 * j - Q0, 0)
                        if jo == 0:
                            a2 = atn.tile([P, 2, 2, 512], f8, tag="a",
                                          bufs=4, name=f"a{b}_{Qq}_{gp}")
                        nc.scalar.activation(a2[:, jo, :, s:512],
                                             sc[:, :, s:512],
                                             AF.Exp,
                                             scale=float(INV_SCALE))
                        if 128 * j >= Q0:
                            for hh in range(2):
                                nc.vector.tensor_mul(
                                    a2[:, jo, hh, s:s + P],
                                    a2[:, jo, hh, s:s + P], triu)
                        if j + 1 < jmax:
                            sc = emit_scores(j + 1)
                        if jo == 1:
                            s0 = max(128 * (j - 1) - Q0, 0)
                            s1 = s
                            for hh in range(2):
                                if s1 > s0:
                                    nc.tensor.matmul(
                                        oT[hh][:, s0:s1],
                                        lhsT=vv[b][:, gp, 0,
                                                   80 * hh:80 * hh + 65],
                                        rhs=a2[:, 0, hh, s0:s1],
                                        start=(gp == 0), stop=False)
                                nc.tensor.matmul(
                                    oT[hh][:, s1:512],
                                    lhsT=vv[b][:, gp, :,
                                               80 * hh:80 * hh + 65],
                                    rhs=a2[:, :, hh, s1:512],
                                    start=(gp == 0),
                                    stop=(gp == 2 * Qq + 1),
                                    perf_mode=DR)
                    for hh in range(2):
                        mi = 2 * b + hh
                        nc.vector.tensor_copy(
                            oT_sbs[mi][:, Q0:Q0 + 512], oT[hh][0:64, :])
                        nc.vector.tensor_copy(
                            dnrows[mi][:, Q0:Q0 + 512], oT[hh][64:65, :])

            def emit_norm(mi):
                b, hh = mi // 2, mi % 2
                oT_sb = oT_sbs.pop(mi)
                dnrow = dnrows.pop(mi)
                dnd_f = dram.tile([1, S], bf16, tag="dndf", bufs=2,
                                  name=f"dndf{mi}")
                nc.sync.dma_start(dnd_f, dnrow)
                wrap = atn.tile([P, 16], bf16, tag="wrap", bufs=2,
                                name=f"wrap{mi}")
                nc.sync.dma_start(
                    wrap, dnd_f[0, :].rearrange("(p f) -> p f", f=16))
                wrap_b = atn.tile([P, 16], bf16, tag="wrapb", bufs=2,
                                  name=f"wrapb{mi}")
                with nc.allow_low_precision(reason="softmax denom recip"):
                    nc.vector.reciprocal(wrap_b, wrap)
                dnd_b = dram.tile([1, S], bf16, tag="dndb", bufs=2,
                                  name=f"dndb{mi}")
                nc.sync.dma_start(
                    dnd_b[0, :].rearrange("(p f) -> p f", f=16), wrap_b)
                rrep = atn.tile([64, S], bf16, tag="rrep", bufs=2,
                                name=f"rrep{mi}")
                nc.sync.dma_start(rrep, dnd_b.to_broadcast([64, S]))
                # normalization folded into the proj-lhsT scatter (fp8 out)
                oT_r = oT_sb.rearrange("d (t a) -> d a t", a=16)
                rr_r = rrep.rearrange("d (t a) -> d a t", a=16)
                for kc in range(8):
                    for ah in range(2):
                        nc.vector.tensor_mul(
                            hT[64 * ah:64 * ah + 64, mi, kc // 2, kc % 2, :],
                            oT_r[:, 2 * kc + ah, :],
                            rr_r[:, 2 * kc + ah, :])

            def emit_proj(mi):
                b, hh = mi // 2, mi % 2
                xr_sb = xr_sbs[mi]
                r_sb = atn.tile([P, E], f32, tag="r", bufs=2,
                                name=f"r{mi}")
                for ns_i in range(2):
                    ns = slice(ns_i * 512, (ns_i + 1) * 512)
                    pacc = psC.tile([P, 512], f32, tag="pacc", bufs=2,
                                    name=f"pa{mi}_{ns_i}")
                    for g in range(4):
                        nc.tensor.matmul(pacc, lhsT=hT[:, mi, g, :, :],
                                         rhs=wp_sb[:, g, :, ns],
                                         start=(g == 0),
                                         stop=(g == 3),
                                         perf_mode=DR)
                    nc.vector.tensor_add(r_sb[:, ns], pacc,
                                         xr_sb[:, ns])
                stats = atn.tile([P, 2, 6], f32, tag="stats", bufs=2,
                                 name=f"st{mi}")
                for sg in range(2):
                    nc.vector.bn_stats(stats[:, sg, :],
                                       r_sb[:, sg * 512:(sg + 1) * 512])
                mv = atn.tile([P, 2], f32, tag="mv", bufs=2,
                              name=f"mv{mi}")
                nc.vector.bn_aggr(mv, stats)
                nc.scalar.activation(mv[:, 1:2], mv[:, 1:2], AF.Sqrt,
                                     bias=eps_t, scale=1.0)
                nc.vector.reciprocal(mv[:, 1:2], mv[:, 1:2])
                ln_m = atn.tile([P, E], fp16, tag="ln", bufs=2,
                                name=f"ln{mi}")
                nc.vector.tensor_scalar(ln_m, r_sb, mv[:, 0:1],
                                        mv[:, 1:2], ALU.subtract,
                                        ALU.mult)
                for kc in range(8):
                    t_ps = psC.tile([P, 1024], fp16, tag="tpl", bufs=2,
                                    name=f"tpl{mi}_{kc}")
                    nc.tensor.transpose(t_ps[:, 0:P],
                                        ln_m[:, P * kc:P * (kc + 1)],
                                        ident_h)
                    nc.vector.tensor_copy(lnT[:, kc, mi, :], t_ps[:, 0:P])

            emit_pair(0)
            nc.sync.dma_start(w1_sb[:, 2:5, :], w1_d[:, 2:5, :])
            emit_norm(0)
            emit_norm(1)
            emit_pair(1)
            nc.sync.dma_start(w1_sb[:, 5:8, :], w1_d[:, 5:8, :])
            psB_cm.__exit__(None, None, None)
            # proj (4 banks) + FFN1 accs (2 banks) coexist: FFN1 is split by
            # mi-pair (N=256) and interleaved with the proj emissions so the
            # PE never idles >3.4us here (HAM stays at full clock).
            psC_cm = tc.tile_pool(name="psC", bufs=1, space="PSUM")
            psC = psC_cm.__enter__()
            psf_cm = tc.tile_pool(name="psf", bufs=1, space="PSUM")
            psf = psf_cm.__enter__()

            def emit_ffn1(mp):
                for fc in range(32):
                    facc = psf.tile([P, 2, P], f32, tag="facc", bufs=4,
                                    name=f"fa{mp}_{fc}")
                    for kc in range(8):
                        nc.tensor.matmul(
                            facc,
                            lhsT=w1_sb[:, kc, P * fc:P * (fc + 1)],
                            rhs=lnT[:, kc, 2 * mp:2 * mp + 2, :],
                            start=(kc == 0), stop=(kc == 7))
                    nc.scalar.activation(h1T[:, fc, 2 * mp:2 * mp + 2, :],
                                         facc, AF.Relu,
                                         bias=b1T_sb[:, fc:fc + 1])

            emit_norm(2)
            emit_proj(0)
            emit_norm(3)
            emit_proj(1)
            emit_ffn1(0)
            emit_proj(2)
            emit_proj(3)
            emit_ffn1(1)
            psf_cm.__exit__(None, None, None)
            psC_cm.__exit__(None, None, None)
            atn_cm.__exit__(None, None, None)
            wppool_cm.__exit__(None, None, None)

            # ---------------- Phase D: FFN2 -------------------------------
            # w2 slabs on the gpsimd queue: the engine reaches these right
            # after the xr loads, so they overlap attention/proj/FFN1 (the
            # SBUF region reuse fences them behind the atn pool's last use).
            w2pool_cm = tc.tile_pool(name="w2pool", bufs=1, side="right")
            w2pool = w2pool_cm.__enter__()
            w2_sb = w2pool.tile([P, 32, E], fp16)
            for g in range(4):
                nc.gpsimd.dma_start(w2_sb[:, 8 * g:8 * g + 8, :],
                                    w2_d[:, 8 * g:8 * g + 8, :])
            pso_cm = tc.tile_pool(name="pso", bufs=1, space="PSUM")
            pso = pso_cm.__enter__()
            for mi in range(4):
                b, hh = mi // 2, mi % 2
                for ns_i in range(2):
                    ns = slice(ns_i * 512, (ns_i + 1) * 512)
                    oacc = pso.tile([P, 512], f32, tag="oacc", bufs=4,
                                    name=f"oa{mi}_{ns_i}")
                    nc.tensor.matmul(oacc, lhsT=ones_h,
                                     rhs=b2_row[:, ns], start=True,
                                     stop=False)
                    for kcf in range(32):
                        nc.tensor.matmul(
                            oacc,
                            lhsT=h1T[:, kcf, mi, :],
                            rhs=w2_sb[:, kcf, ns],
                            start=False, stop=(kcf == 31))
                    o_half = w2pool.tile([P, 512], f32, tag="o", bufs=4,
                                         name=f"o{mi}_{ns_i}")
                    nc.vector.tensor_copy(o_half, oacc)
                    nc.gpsimd.dma_start(out_d[b, hh, :, ns], o_half)
            pso_cm.__exit__(None, None, None)
            w2pool_cm.__exit__(None, None, None)
            midpool_cm.__exit__(None, None, None)
            w1pool_cm.__exit__(None, None, None)

    nc.compile()
    return nc


def _get_nc():
    if "nc" not in _cached:
        _cached["nc"] = _build()
    return _cached["nc"]


def _make_in_maps(inputs):
    import ml_dtypes
    bf = ml_dtypes.bfloat16
    e4 = ml_dtypes.float8_e4m3
    x = np.ascontiguousarray(np.asarray(inputs["x"], dtype=np.float32))
    w_qkv = np.asarray(inputs["w_qkv"], dtype=np.float32)
    b_qkv = np.asarray(inputs["b_qkv"], dtype=np.float32)
    w_proj = np.asarray(inputs["w_proj"], dtype=np.float32)
    b_proj = np.asarray(inputs["b_proj"], dtype=np.float32)
    ln_g = np.asarray(inputs["ln_g"], dtype=np.float32)
    ln_b = np.asarray(inputs["ln_b"], dtype=np.float32)
    w1 = np.asarray(inputs["w1"], dtype=np.float32)
    b1 = np.asarray(inputs["b1"], dtype=np.float32)
    w2 = np.asarray(inputs["w2"], dtype=np.float32)
    b2 = np.asarray(inputs["b2"], dtype=np.float32)

    w1e = ln_g[:, None] * w1                     # [E, FF]
    b1e = b1 + ln_b @ w1                         # [FF]

    wq_h = np.ascontiguousarray(
        w_qkv.reshape(4, 2, P, ROW).transpose(2, 0, 1, 3)).astype(e4)
    wp_h = np.ascontiguousarray(
        w_proj.reshape(4, 2, P, E).transpose(2, 0, 1, 3)).astype(e4)
    w1_h = np.ascontiguousarray(
        w1e.reshape(8, P, FF).transpose(1, 0, 2)).astype(np.float16)
    w2_h = np.ascontiguousarray(
        w2.reshape(32, P, E).transpose(1, 0, 2)).astype(np.float16)
    b1T_h = np.ascontiguousarray(b1e.reshape(32, P).T).astype(np.float32)
    bq_h = b_qkv.reshape(1, ROW).astype(bf)
    b2_h = b2.reshape(1, E).astype(np.float16)
    triu_h = np.triu(np.ones((P, P))).astype(e4)
    ones_h = np.ones((1, P), np.float32)

    in_maps = []
    slots_l = [(b, t) for b in range(B) for t in range(3)]
    for c in range(NCORES):
        xq_full = np.zeros((P, 8, 544), np.float32)
        offs = np.zeros((1, 4), np.uint32)
        for m, (b, t) in enumerate(slots_l):
            start = (16 * t + 2 * c) * BLK
            T0 = start // ROW
            offs[0, t] = ROW - (start - T0 * ROW)
            n = min(88, S - T0)
            xs = x[b, T0:T0 + n]                 # [n, E]
            xq_full[:, :, 88 * m:88 * m + n] = np.ascontiguousarray(
                xs.T).reshape(8, P, n).transpose(1, 0, 2)
        xqT = xq_full.reshape(P, 4, 2, 544).astype(e4)
        xr = np.zeros((B, 2, P, E), np.float32)
        for hh in range(2):
            h_ = 2 * c + hh
            for b in range(B):
                xr[b, hh] = x[b, P * h_:P * (h_ + 1)] + b_proj
        in_maps.append({
            "xqT": xqT, "xr": xr, "offs": offs,
            "ones": ones_h, "triu": triu_h,
            "wq": wq_h, "bq": bq_h, "wp": wp_h,
            "w1": w1_h, "b1T": b1T_h, "w2": w2_h, "b2": b2_h,
        })
    return in_maps


def _run(inputs, trace=False, trace_cores=None):
    import sys
    if "/opt/trn_rl_repo" not in sys.path:
        sys.path.insert(0, "/opt/trn_rl_repo")
    from concourse.bass_utils import run_bass_kernel_spmd
    nc = _get_nc()
    in_maps = _make_in_maps(inputs)
    kwargs = {}
    if trace:
        kwargs["trace"] = True
        if trace_cores is not None:
            kwargs["trace_cores"] = trace_cores
    res = run_bass_kernel_spmd(nc, in_maps, list(range(NCORES)), **kwargs)
    full = np.zeros((B, S, E), np.float32)
    for c in range(NCORES):
        o = res.results[c]["out"]
        for hh in range(2):
            h_ = 2 * c + hh
            for b in range(B):
                full[b, P * h_:P * (h_ + 1)] = o[b, hh]
    return full, res


def kernel(**inputs) -> np.ndarray:
    import sys
    if "/opt/trn_rl_repo" not in sys.path:
        sys.path.insert(0, "/opt/trn_rl_repo")
    full, _ = _run(inputs)
    return full


# revision 36
# speedup vs baseline: 1.0462x; 1.0321x over previous
# Trainium2 Bass kernel for nn_MultiHeadTransformer (B=2, S=2048, E=1024, H=16, FF=4096).
#
# Sharding: 8-way head/row parallel with zero collectives (the reference's
# "faithful raw view" makes qkv/attention/proj/LN/FFN row-local per core: core c
# computes the qkv rows covering the six flat (type, head) blocks of its two
# heads; the inverse view maps head outputs back to its own 256 token rows).
#
# v5: fp8-e4m3 + DoubleRow on QKV / AV / proj (validated end-to-end rel err
# ~4e-3 vs the 2e-2 gate in a numpy quantization sim; FFN stays fp16 because
# fp8 there alone costs ~4e-2):
#  - QKV: x^T and w_qkv host-packed as [P, 4, 2, *] fp8 kc-pairs; per-mt waves
#    of 6 PSUM accs, 4 DoubleRow passes each; per-g contiguous DRAM slabs so
#    the first matmul starts as soon as slab g0 lands.
#  - scratch (qkv scatter through DRAM) in fp8: half the roundtrip bytes.
#  - q/k staged fp8, PE-transposed fp8, converted to fp8 qT/kT via the
#    existing DVE copy-out; scores run fp8 (same PE rate as bf16) with the
#    two heads' score matmuls emitted interleaved on disjoint PE row halves
#    (rows 0-63 / 64-127) so they execute concurrently.
#  - AV: v staged as [P, 8, 2, 160] fp8 j-pairs (ones row at +64 for the
#    softmax denominator); exp writes fp8 into paired a2 tiles; AV runs one
#    DoubleRow matmul per j-pair, plus a single-j matmul for the causal
#    straddle chunk [s_even, s_odd).
#  - proj: hT scatter written fp8 in kc-pairs, w_proj fp8 [P, 4, 2, E],
#    4 DoubleRow passes per 512-col chunk.
#  - causal masks multiplied on DVE (fp8); gpsimd keeps only scratch writes,
#    xr loads and output writes; output written per 512-col half.
import numpy as np

B, S, E, H, DH, FF = 2, 2048, 1024, 16, 64, 4096
ROW = 3 * E            # 3072 qkv columns
BLK = S * DH           # 131072 elements per (type, head) block
NCORES = 8
P = 128
INV_SCALE = 1.0 / float(np.sqrt(E))

_cached = {}


def _build():
    import concourse.bacc as bacc
    import concourse.bass as bass
    import concourse.mybir as mybir
    import concourse.tile as tile
    from concourse.masks import make_identity

    f32 = mybir.dt.float32
    bf16 = mybir.dt.bfloat16
    fp16 = mybir.dt.float16
    f8 = mybir.dt.float8e4
    u32 = mybir.dt.uint32
    AF = mybir.ActivationFunctionType
    ALU = mybir.AluOpType
    DR = mybir.MatmulPerfMode.DoubleRow

    nc = bacc.Bacc(trn_type="TRN2", target_bir_lowering=False, debug=False,
                   num_devices=NCORES)

    xqT_d = nc.dram_tensor("xqT", [P, 4, 2, 544], f8,
                           kind="ExternalInput").ap()
    wq_d = nc.dram_tensor("wq", [P, 4, 2, ROW], f8, kind="ExternalInput").ap()
    bq_d = nc.dram_tensor("bq", [1, ROW], bf16, kind="ExternalInput").ap()
    wp_d = nc.dram_tensor("wp", [P, 4, 2, E], f8, kind="ExternalInput").ap()
    w1_d = nc.dram_tensor("w1", [P, 8, FF], fp16, kind="ExternalInput").ap()
    b1T_d = nc.dram_tensor("b1T", [P, 32], f32, kind="ExternalInput").ap()
    w2_d = nc.dram_tensor("w2", [P, 32, E], fp16, kind="ExternalInput").ap()
    b2_d = nc.dram_tensor("b2", [1, E], fp16, kind="ExternalInput").ap()
    xr_d = nc.dram_tensor("xr", [B, 2, P, E], f32, kind="ExternalInput").ap()
    offs_d = nc.dram_tensor("offs", [1, 4], u32, kind="ExternalInput").ap()
    triu_d = nc.dram_tensor("triu", [P, P], f8, kind="ExternalInput").ap()
    ones_d = nc.dram_tensor("ones", [1, P], f32, kind="ExternalInput").ap()
    out_d = nc.dram_tensor("out", [B, 2, P, E], f32, kind="ExternalOutput").ap()

    slots = [(b, t) for b in range(B) for t in range(3)]

    with tile.TileContext(nc) as tc:
        with tc.tile_pool(name="singles", bufs=1) as singles, \
             tc.tile_pool(name="dram", bufs=1, space="DRAM") as dram:
            triu = singles.tile([P, P], f8)
            b2_row = singles.tile([1, E], fp16)
            b1T_sb = singles.tile([P, 32], f32)
            ones_f = singles.tile([1, P], f32)
            nc.sync.dma_start(ones_f, ones_d)
            ones_h = singles.tile([1, P], fp16)
            nc.vector.tensor_copy(ones_h, ones_f)
            ident_b = singles.tile([P, P], bf16)
            make_identity(nc, ident_b)
            ident_8 = singles.tile([P, P], f8)
            nc.vector.tensor_copy(ident_8, ident_b)
            ident_h = singles.tile([P, P], fp16)
            make_identity(nc, ident_h)
            eps_t = singles.tile([P, 1], f32)
            nc.vector.memset(eps_t, 1e-5)
            offs_sb = singles.tile([1, 4], u32)
            nc.sync.dma_start(offs_sb, offs_d)
            # offsets are only used by gpsimd (Pool) scratch-write DMAs;
            # loading on all engines costs a ~5us all-engine barrier preamble.
            off_v = [nc.values_load(offs_sb[:, t:t + 1],
                                    engines=[mybir.EngineType.Pool],
                                    min_val=0, max_val=ROW,
                                    skip_runtime_bounds_check=True)
                     for t in range(3)]

            SCR88 = ROW + 88 * ROW
            scr = [[dram.tile([SCR88], f8, tag=f"scr{b}{t}",
                              name=f"scr{b}_{t}") for t in range(3)]
                   for b in range(B)]

            # PSUM pools are phase-scoped: psA (QKV accs + q/k transposes)
            # -> psB (attention sc+oT, 8 banks) -> psC (proj accs).
            psA_cm = tc.tile_pool(name="psA", bufs=1, space="PSUM")
            psA = psA_cm.__enter__()
            midpool_cm = tc.tile_pool(name="midpool", bufs=1, side="right")
            midpool = midpool_cm.__enter__()
            hT = midpool.tile([P, 4, 4, 2, P], f8)
            lnT = midpool.tile([P, 8, 4, P], fp16)
            h1T = midpool.tile([P, 32, 4, P], fp16)
            atn_cm = tc.tile_pool(name="atn", bufs=1, side="right")
            atn = atn_cm.__enter__()

            qs8, ks8, vv, qT, kT = {}, {}, {}, {}, {}

            def emit_read(b):
                # merged scratch reads: one DMA per tensor (q/k/v), both
                # heads, all 16 token blocks.  Issued on sync.
                qs8[b] = atn.tile([P, 16, P], f8, tag="qstage", bufs=2,
                                  name=f"qs{b}")
                ks8[b] = atn.tile([P, 16, P], f8, tag="kstage", bufs=2,
                                  name=f"ks{b}")
                vv[b] = atn.tile([P, 8, 2, 160], f8, tag="v", bufs=2,
                                 name=f"v{b}")
                nc.vector.memset(vv[b][:, :, :, 64:65], 1.0)
                nc.vector.memset(vv[b][:, :, :, 144:145], 1.0)
                def src_of(t, hh):
                    return (scr[b][t][ROW + hh * BLK:ROW + (hh + 1) * BLK]
                            .rearrange("(i p d) -> p i d", p=P, d=DH))
                # q first (its transposes gate the first scores), then k, v
                for hh in range(2):
                    nc.sync.dma_start(
                        qs8[b][:, :, 64 * hh:64 * hh + 64], src_of(0, hh))
                for hh in range(2):
                    nc.sync.dma_start(
                        ks8[b][:, :, 64 * hh:64 * hh + 64], src_of(1, hh))
                for hh in range(2):
                    vdst = vv[b][:, :, :, 80 * hh:80 * hh + 64]
                    nc.sync.dma_start(
                        vdst.rearrange("p g j d -> p (g j) d"), src_of(2, hh))
                qT[b] = atn.tile([P, S], f8, tag="qT", bufs=2,
                                 name=f"qT{b}")
                kT[b] = atn.tile([P, S], f8, tag="kT", bufs=2,
                                 name=f"kT{b}")

            def emit_tp(b):
                # all q transposes first: the first scores only need qT+kT[0]
                for st, dst in ((qs8[b], qT[b]), (ks8[b], kT[b])):
                    for i in range(16):
                        # fp8 PE transpose writes 2-byte slots: use a
                        # stride-2 output AP (low byte of each pair).
                        t_ps = psA.tile([P, 1024, 2], f8, tag="tp", bufs=2,
                                        name=f"tp{b}_{i}")
                        nc.tensor.transpose(t_ps[:, 0:P, 0], st[:, i, :],
                                            ident_8)
                        nc.vector.tensor_copy(dst[:, P * i:P * (i + 1)],
                                              t_ps[:, 0:P, 0])

            # ---------------- Phase A: QKV fp8 DoubleRow ------------------
            poolA_cm = tc.tile_pool(name="poolA", bufs=1)
            poolA = poolA_cm.__enter__()
            wq_sb = poolA.tile([P, 4, 2, ROW], f8)
            xqT_sb = poolA.tile([P, 4, 2, 544], f8)
            bias_bc = poolA.tile([P, ROW], bf16)
            # per-g slabs, g0 first so matmuls start ~3us in; wq slab split
            # in halves so wave g can begin on its first columns.
            for g in range(4):
                nc.sync.dma_start(xqT_sb[:, g], xqT_d[:, g])
                if g == 0:
                    # finer slabs: the first matmul only needs cols 0:512
                    for c6 in range(6):
                        cs = slice(512 * c6, 512 * c6 + 512)
                        nc.sync.dma_start(wq_sb[:, 0, :, cs],
                                          wq_d[:, 0, :, cs])
                else:
                    nc.sync.dma_start(wq_sb[:, g, :, 0:1536],
                                      wq_d[:, g, :, 0:1536])
                    nc.sync.dma_start(wq_sb[:, g, :, 1536:ROW],
                                      wq_d[:, g, :, 1536:ROW])
            nc.sync.dma_start(bias_bc, bq_d.to_broadcast([P, ROW]))
            nc.sync.dma_start(triu, triu_d)
            nc.sync.dma_start(b2_row, b2_d)
            nc.sync.dma_start(b1T_sb, b1T_d)

            # 528 packed rows -> 5 M-tiles of <=128; slot m=(b,t) owns
            # global rows [88m, 88m+88).
            yts = {}
            for mt in range(5):
                g0, g1 = 128 * mt, min(128 * mt + 128, 528)
                M = g1 - g0
                y = poolA.tile([P, ROW], f8, tag="y", bufs=2,
                               name=f"y{mt}")
                yts[mt] = y
                accs = [psA.tile([P, 512], f32, tag="acc", bufs=6,
                                 name=f"qa{mt}_{n6}") for n6 in range(6)]
                for g in range(4):
                    for n6 in range(6):
                        ns = slice(n6 * 512, (n6 + 1) * 512)
                        nc.tensor.matmul(accs[n6][:M, :],
                                         lhsT=xqT_sb[:, g, :, g0:g1],
                                         rhs=wq_sb[:, g, :, ns],
                                         start=(g == 0), stop=(g == 3),
                                         perf_mode=DR)
                for n6 in range(6):
                    ns = slice(n6 * 512, (n6 + 1) * 512)
                    nc.vector.tensor_add(y[:M, ns], accs[n6][:M, :],
                                         bias_bc[:M, ns])
                for m, (b, t) in enumerate(slots):
                    s0, s1 = 88 * m, 88 * m + 88
                    if mt != (s1 - 1) // 128:
                        continue
                    # rows of this slot, split by containing tile
                    spans = []
                    for tt in (mt - 1, mt):
                        if tt < 0:
                            continue
                        lo = max(s0, 128 * tt)
                        hi = min(s1, 128 * tt + 128)
                        if lo < hi:
                            spans.append((tt, lo, hi))
                    dst = scr[b][t][bass.ds(off_v[t], 88 * ROW)]
                    dst_r = dst.rearrange("(r c) -> r c", c=ROW)
                    for tt, lo, hi in spans:
                        nc.gpsimd.dma_start(
                            dst_r[lo - s0:hi - s0, :],
                            yts[tt][lo - 128 * tt:hi - 128 * tt, :])
                    if m == 2:
                        emit_read(0)
            poolA_cm.__exit__(None, None, None)

            w1pool_cm = tc.tile_pool(name="w1pool", bufs=1)
            w1pool = w1pool_cm.__enter__()
            w1_sb = w1pool.tile([P, 8, FF], fp16)
            wppool_cm = tc.tile_pool(name="wppool", bufs=1)
            wppool = wppool_cm.__enter__()
            wp_sb = wppool.tile([P, 4, 2, E], f8)
            emit_read(1)
            emit_tp(0)
            emit_tp(1)
            nc.sync.dma_start(wp_sb, wp_d)
            nc.sync.dma_start(w1_sb[:, 0:2, :], w1_d[:, 0:2, :])
            # residual inputs preloaded on the gpsimd queue (idle here) so
            # they are never stuck behind the w2 slabs later.
            xr_sbs = {}
            for mi in range(4):
                xr_sbs[mi] = atn.tile([P, E], f32, tag="xr", bufs=4,
                                      name=f"xr{mi}")
                nc.gpsimd.dma_start(xr_sbs[mi], xr_d[mi // 2, mi % 2])
            psA_cm.__exit__(None, None, None)
            psB_cm = tc.tile_pool(name="psB", bufs=1, space="PSUM")
            psB = psB_cm.__enter__()

            # ------------- Phase B+C: attention / proj / LN ---------------
            # Query-quarter blocks (512 queries): both heads' scores share
            # one 2-bank PSUM tile, so exp runs ONCE per j for both heads
            # (halves ACT instruction overhead); sc bufs=3 gives a 2-j
            # lookahead so ACT never starves; AV per j-pair via DoubleRow.
            oT_sbs, dnrows = {}, {}

            def emit_pair(b):
                for hh in range(2):
                    mi = 2 * b + hh
                    oT_sbs[mi] = atn.tile([64, S], bf16, tag="oTsb",
                                          bufs=2, name=f"oTsb{mi}")
                    dnrows[mi] = atn.tile([1, S], bf16, tag="dnrow",
                                          bufs=2, name=f"dn{mi}")

                for Qq in range(4):
                    Q0 = 512 * Qq
                    jmax = 4 * (Qq + 1)
                    oT = {hh: psB.tile([65, 512], f32, tag="oT", bufs=2,
                                       name=f"oT{b}{hh}_{Qq}")
                          for hh in range(2)}

                    def emit_scores(j):
                        s = max(128 * j - Q0, 0)
                        t_sc = psB.tile([P, 2, 512], f32, tag="sc",
                                        bufs=3, name=f"sc{b}_{Qq}_{j}")
                        for hh in range(2):
                            hp = slice(64 * hh, 64 * hh + 64)
                            nc.tensor.matmul(
                                t_sc[:, hh, s:512],
                                lhsT=kT[b][hp, P * j:P * (j + 1)],
                                rhs=qT[b][hp, Q0 + s:Q0 + 512],
                                start=True, stop=True)
                        return t_sc

                    sc = emit_scores(0)
                    a2 = None
                    for j in range(jmax):
                        gp, jo = j // 2, j % 2
                        s = max(128 * j - Q0, 0)
                        if jo == 0:
                            a2 = atn.tile([P, 2, 2, 512], f8, tag="a",
                                          bufs=4, name=f"a{b}_{Qq}_{gp}")
                        nc.scalar.activation(a2[:, jo, :, s:512],
                                             sc[:, :, s:512],
                                             AF.Exp,
                                             scale=float(INV_SCALE))
                        if 128 * j >= Q0:
                            for hh in range(2):
                                nc.vector.tensor_mul(
                                    a2[:, jo, hh, s:s + P],
                                    a2[:, jo, hh, s:s + P], triu)
                        if j + 1 < jmax:
                            sc = emit_scores(j + 1)
                        if jo == 1:
                            s0 = max(128 * (j - 1) - Q0, 0)
                            s1 = s
                            for hh in range(2):
                                if s1 > s0:
                                    nc.tensor.matmul(
                                        oT[hh][:, s0:s1],
                                        lhsT=vv[b][:, gp, 0,
                                                   80 * hh:80 * hh + 65],
                                        rhs=a2[:, 0, hh, s0:s1],
                                        start=(gp == 0), stop=False)
                                nc.tensor.matmul(
                                    oT[hh][:, s1:512],
                                    lhsT=vv[b][:, gp, :,
                                               80 * hh:80 * hh + 65],
                                    rhs=a2[:, :, hh, s1:512],
                                    start=(gp == 0),
                                    stop=(gp == 2 * Qq + 1),
                                    perf_mode=DR)
                    for hh in range(2):
                        mi = 2 * b + hh
                        nc.vector.tensor_copy(
                            oT_sbs[mi][:, Q0:Q0 + 512], oT[hh][0:64, :])
                        nc.vector.tensor_copy(
                            dnrows[mi][:, Q0:Q0 + 512], oT[hh][64:65, :])

            def emit_norm(mi):
                b, hh = mi // 2, mi % 2
                oT_sb = oT_sbs.pop(mi)
                dnrow = dnrows.pop(mi)
                dnd_f = dram.tile([1, S], bf16, tag="dndf", bufs=2,
                                  name=f"dndf{mi}")
                nc.sync.dma_start(dnd_f, dnrow)
                wrap = atn.tile([P, 16], bf16, tag="wrap", bufs=2,
                                name=f"wrap{mi}")
                nc.sync.dma_start(
                    wrap, dnd_f[0, :].rearrange("(p f) -> p f", f=16))
                wrap_b = atn.tile([P, 16], bf16, tag="wrapb", bufs=2,
                                  name=f"wrapb{mi}")
                with nc.allow_low_precision(reason="softmax denom recip"):
                    nc.vector.reciprocal(wrap_b, wrap)
                dnd_b = dram.tile([1, S], bf16, tag="dndb", bufs=2,
                                  name=f"dndb{mi}")
                nc.sync.dma_start(
                    dnd_b[0, :].rearrange("(p f) -> p f", f=16), wrap_b)
                rrep = atn.tile([64, S], bf16, tag="rrep", bufs=2,
                                name=f"rrep{mi}")
                nc.sync.dma_start(rrep, dnd_b.to_broadcast([64, S]))
                # normalization folded into the proj-lhsT scatter (fp8 out)
                oT_r = oT_sb.rearrange("d (t a) -> d a t", a=16)
                rr_r = rrep.rearrange("d (t a) -> d a t", a=16)
                for kc in range(8):
                    for ah in range(2):
                        nc.vector.tensor_mul(
                            hT[64 * ah:64 * ah + 64, mi, kc // 2, kc % 2, :],
                            oT_r[:, 2 * kc + ah, :],
                            rr_r[:, 2 * kc + ah, :])

            def emit_proj(mi):
                b, hh = mi // 2, mi % 2
                xr_sb = xr_sbs[mi]
                r_sb = atn.tile([P, E], f32, tag="r", bufs=2,
                                name=f"r{mi}")
                for ns_i in range(2):
                    ns = slice(ns_i * 512, (ns_i + 1) * 512)
                    pacc = psC.tile([P, 512], f32, tag="pacc", bufs=2,
                                    name=f"pa{mi}_{ns_i}")
                    for g in range(4):
                        nc.tensor.matmul(pacc, lhsT=hT[:, mi, g, :, :],
                                         rhs=wp_sb[:, g, :, ns],
                                         start=(g == 0),
                                         stop=(g == 3),
                                         perf_mode=DR)
                    nc.vector.tensor_add(r_sb[:, ns], pacc,
                                         xr_sb[:, ns])
                stats = atn.tile([P, 2, 6], f32, tag="stats", bufs=2,
                                 name=f"st{mi}")
                for sg in range(2):
                    nc.vector.bn_stats(stats[:, sg, :],
                                       r_sb[:, sg * 512:(sg + 1) * 512])
                mv = atn.tile([P, 2], f32, tag="mv", bufs=2,
                              name=f"mv{mi}")
                nc.vector.bn_aggr(mv, stats)
                nc.scalar.activation(mv[:, 1:2], mv[:, 1:2], AF.Sqrt,
                                     bias=eps_t, scale=1.0)
                nc.vector.reciprocal(mv[:, 1:2], mv[:, 1:2])
                ln_m = atn.tile([P, E], fp16, tag="ln", bufs=2,
                                name=f"ln{mi}")
                nc.vector.tensor_scalar(ln_m, r_sb, mv[:, 0:1],
                                        mv[:, 1:2], ALU.subtract,
                                        ALU.mult)
                for kc in range(8):
                    t_ps = psC.tile([P, 1024], fp16, tag="tpl", bufs=2,
                                    name=f"tpl{mi}_{kc}")
                    nc.tensor.transpose(t_ps[:, 0:P],
                                        ln_m[:, P * kc:P * (kc + 1)],
                                        ident_h)
                    nc.vector.tensor_copy(lnT[:, kc, mi, :], t_ps[:, 0:P])

            emit_pair(0)
            nc.sync.dma_start(w1_sb[:, 2:5, :], w1_d[:, 2:5, :])
            emit_norm(0)
            emit_norm(1)
            emit_pair(1)
            nc.sync.dma_start(w1_sb[:, 5:8, :], w1_d[:, 5:8, :])
            psB_cm.__exit__(None, None, None)
            # proj (4 banks) + FFN1 accs (2 banks) coexist: FFN1 is split by
            # mi-pair (N=256) and interleaved with the proj emissions so the
            # PE never idles >3.4us here (HAM stays at full clock).
            psC_cm = tc.tile_pool(name="psC", bufs=1, space="PSUM")
            psC = psC_cm.__enter__()
            psf_cm = tc.tile_pool(name="psf", bufs=1, space="PSUM")
            psf = psf_cm.__enter__()

            def emit_ffn1(mp):
                for fc in range(32):
                    facc = psf.tile([P, 2, P], f32, tag="facc", bufs=4,
                                    name=f"fa{mp}_{fc}")
                    for kc in range(8):
                        nc.tensor.matmul(
                            facc,
                            lhsT=w1_sb[:, kc, P * fc:P * (fc + 1)],
                            rhs=lnT[:, kc, 2 * mp:2 * mp + 2, :],
                            start=(kc == 0), stop=(kc == 7))
                    nc.scalar.activation(h1T[:, fc, 2 * mp:2 * mp + 2, :],
                                         facc, AF.Relu,
                                         bias=b1T_sb[:, fc:fc + 1])

            emit_norm(2)
            emit_norm(3)
            emit_proj(0)
            emit_proj(1)
            emit_proj(2)
            emit_proj(3)
            emit_ffn1(0)
            emit_ffn1(1)
            psf_cm.__exit__(None, None, None)
            psC_cm.__exit__(None, None, None)
            atn_cm.__exit__(None, None, None)
            wppool_cm.__exit__(None, None, None)

            # ---------------- Phase D: FFN2 -------------------------------
            # w2 slabs on the gpsimd queue: the engine reaches these right
            # after the xr loads, so they overlap attention/proj/FFN1 (the
            # SBUF region reuse fences them behind the atn pool's last use).
            w2pool_cm = tc.tile_pool(name="w2pool", bufs=1, side="right")
            w2pool = w2pool_cm.__enter__()
            w2_sb = w2pool.tile([P, 32, E], fp16)
            for g in range(4):
                nc.gpsimd.dma_start(w2_sb[:, 8 * g:8 * g + 8, :],
                                    w2_d[:, 8 * g:8 * g + 8, :])
            pso_cm = tc.tile_pool(name="pso", bufs=1, space="PSUM")
            pso = pso_cm.__enter__()
            for mi in range(4):
                b, hh = mi // 2, mi % 2
                for ns_i in range(2):
                    ns = slice(ns_i * 512, (ns_i + 1) * 512)
                    oacc = pso.tile([P, 512], f32, tag="oacc", bufs=4,
                                    name=f"oa{mi}_{ns_i}")
                    nc.tensor.matmul(oacc, lhsT=ones_h,
                                     rhs=b2_row[:, ns], start=True,
                                     stop=False)
                    for kcf in range(32):
                        nc.tensor.matmul(
                            oacc,
                            lhsT=h1T[:, kcf, mi, :],
                            rhs=w2_sb[:, kcf, ns],
                            start=False, stop=(kcf == 31))
                    o_half = w2pool.tile([P, 512], f32, tag="o", bufs=4,
                                         name=f"o{mi}_{ns_i}")
                    nc.vector.tensor_copy(o_half, oacc)
                    nc.gpsimd.dma_start(out_d[b, hh, :, ns], o_half)
            pso_cm.__exit__(None, None, None)
            w2pool_cm.__exit__(None, None, None)
            midpool_cm.__exit__(None, None, None)
            w1pool_cm.__exit__(None, None, None)

    nc.compile()
    return nc


def _get_nc():
    if "nc" not in _cached:
        _cached["nc"] = _build()
    return _cached["nc"]


def _make_in_maps(inputs):
    import ml_dtypes
    bf = ml_dtypes.bfloat16
    e4 = ml_dtypes.float8_e4m3
    x = np.ascontiguousarray(np.asarray(inputs["x"], dtype=np.float32))
    w_qkv = np.asarray(inputs["w_qkv"], dtype=np.float32)
    b_qkv = np.asarray(inputs["b_qkv"], dtype=np.float32)
    w_proj = np.asarray(inputs["w_proj"], dtype=np.float32)
    b_proj = np.asarray(inputs["b_proj"], dtype=np.float32)
    ln_g = np.asarray(inputs["ln_g"], dtype=np.float32)
    ln_b = np.asarray(inputs["ln_b"], dtype=np.float32)
    w1 = np.asarray(inputs["w1"], dtype=np.float32)
    b1 = np.asarray(inputs["b1"], dtype=np.float32)
    w2 = np.asarray(inputs["w2"], dtype=np.float32)
    b2 = np.asarray(inputs["b2"], dtype=np.float32)

    w1e = ln_g[:, None] * w1                     # [E, FF]
    b1e = b1 + ln_b @ w1                         # [FF]

    wq_h = np.ascontiguousarray(
        w_qkv.reshape(4, 2, P, ROW).transpose(2, 0, 1, 3)).astype(e4)
    wp_h = np.ascontiguousarray(
        w_proj.reshape(4, 2, P, E).transpose(2, 0, 1, 3)).astype(e4)
    w1_h = np.ascontiguousarray(
        w1e.reshape(8, P, FF).transpose(1, 0, 2)).astype(np.float16)
    w2_h = np.ascontiguousarray(
        w2.reshape(32, P, E).transpose(1, 0, 2)).astype(np.float16)
    b1T_h = np.ascontiguousarray(b1e.reshape(32, P).T).astype(np.float32)
    bq_h = b_qkv.reshape(1, ROW).astype(bf)
    b2_h = b2.reshape(1, E).astype(np.float16)
    triu_h = np.triu(np.ones((P, P))).astype(e4)
    ones_h = np.ones((1, P), np.float32)

    in_maps = []
    slots_l = [(b, t) for b in range(B) for t in range(3)]
    for c in range(NCORES):
        xq_full = np.zeros((P, 8, 544), np.float32)
        offs = np.zeros((1, 4), np.uint32)
        for m, (b, t) in enumerate(slots_l):
            start = (16 * t + 2 * c) * BLK
            T0 = start // ROW
            offs[0, t] = ROW - (start - T0 * ROW)
            n = min(88, S - T0)
            xs = x[b, T0:T0 + n]                 # [n, E]
            xq_full[:, :, 88 * m:88 * m + n] = np.ascontiguousarray(
                xs.T).reshape(8, P, n).transpose(1, 0, 2)
        xqT = xq_full.reshape(P, 4, 2, 544).astype(e4)
        xr = np.zeros((B, 2, P, E), np.float32)
        for hh in range(2):
            h_ = 2 * c + hh
            for b in range(B):
                xr[b, hh] = x[b, P * h_:P * (h_ + 1)] + b_proj
        in_maps.append({
            "xqT": xqT, "xr": xr, "offs": offs,
            "ones": ones_h, "triu": triu_h,
            "wq": wq_h, "bq": bq_h, "wp": wp_h,
            "w1": w1_h, "b1T": b1T_h, "w2": w2_h, "b2": b2_h,
        })
    return in_maps


def _run(inputs, trace=False, trace_cores=None):
    import sys
    if "/opt/trn_rl_repo" not in sys.path:
        sys.path.insert(0, "/opt/trn_rl_repo")
    from concourse.bass_utils import run_bass_kernel_spmd
    nc = _get_nc()
    in_maps = _make_in_maps(inputs)
    kwargs = {}
    if trace:
        kwargs["trace"] = True
        if trace_cores is not None:
            kwargs["trace_cores"] = trace_cores
    res = run_bass_kernel_spmd(nc, in_maps, list(range(NCORES)), **kwargs)
    full = np.zeros((B, S, E), np.float32)
    for c in range(NCORES):
        o = res.results[c]["out"]
        for hh in range(2):
            h_ = 2 * c + hh
            for b in range(B):
                full[b, P * h_:P * (h_ + 1)] = o[b, hh]
    return full, res


def kernel(**inputs) -> np.ndarray:
    import sys
    if "/opt/trn_rl_repo" not in sys.path:
        sys.path.insert(0, "/opt/trn_rl_repo")
    full, _ = _run(inputs)
    return full
